# revision 1
# baseline (speedup 1.0000x reference)
"""HalfKA NNUE forward pass on 8 Trainium2 NeuronCores.

Network (fp32 reference):
    h1  = relu(x @ W1.T + b1)     x:[2048, 98304] sparse 0/1, W1:[256, 98304]
    h2  = relu(h1 @ W2.T + b2)    W2:[32, 256]
    out = h2 @ Wout.T + bout      Wout:[1, 32]  -> [2048, 1]

Strategy: tensor-parallel over the fc1 contraction (input_dim). Each of the 8
cores reads its own 12288-column slice of x (the dominant 100 MB/core stream)
plus a 12288-row slice of W1, accumulates a partial h1 [256, 2048] on the PE
array, and the partials are summed with an on-device AllReduce. fc2/fc3 are
tiny and computed (redundantly) on every core in fp32.

fc1 streams the PE in bf16 at 1 cycle/row (fp32 matmul is 4 cycles/row);
x is 0/1 so its bf16 cast is exact and only W1's bf16 rounding contributes
error (~5e-4 rel). Set W1_PASSES=2 for a bf16 hi+lo split of W1 (two
accumulating passes, ~1e-6 rel, ~10% slower end-to-end).

The batch is processed in 4 chunks of 512 so each chunk's AllReduce overlaps
the next chunk's fc1 matmuls.
"""

import os
import sys

sys.path.insert(0, "/opt/trn_rl_repo")

from contextlib import ExitStack

import numpy as np
import ml_dtypes

import concourse.bass as bass
import concourse.tile as tile
from concourse import bacc, mybir
from concourse.bass_utils import run_bass_kernel_spmd

f32 = mybir.dt.float32
bf16 = mybir.dt.bfloat16

N_CORES = 8
B = 2048
IN_DIM = 98304
H1 = 256
H2 = 32

P = 128
# fc1 weight passes: 1 = bf16 (rel err ~7e-4), 2 = bf16 hi+lo (rel err ~1e-6)
W1_PASSES = 1
KSH = IN_DIM // N_CORES          # 12288 contraction dims per core
KT = KSH // P                    # 96 k-tiles per core
SUP = 8                          # k-tiles per x DMA super-tile (16 KB/partition)
NSUP = KT // SUP                 # 12
CHUNK = 512                      # batch columns per chunk (one PSUM bank)
NCH = B // CHUNK                 # 4
M_T = H1 // P                    # 2 h1 partition-tiles

_CACHED = {}


def _build_program():
    nc = bacc.Bacc(
        "TRN2",
        target_bir_lowering=False,
        debug=False,
        num_devices=N_CORES,
    )

    xt = nc.dram_tensor("xt", [NCH, P, KT // SUP, SUP, CHUNK], f32, kind="ExternalInput")
    w1hi = nc.dram_tensor("w1hi", [P, KT, H1], bf16, kind="ExternalInput")
    w1lo = (
        nc.dram_tensor("w1lo", [P, KT, H1], bf16, kind="ExternalInput")
        if W1_PASSES == 2 else None
    )
    b1 = nc.dram_tensor("b1", [P, M_T], f32, kind="ExternalInput")
    w2t = nc.dram_tensor("w2t", [P, M_T, H2], f32, kind="ExternalInput")
    b2 = nc.dram_tensor("b2", [H2, 1], f32, kind="ExternalInput")
    # [Wout.T; bout] stacked: fc3 computes Wout @ h2 + bout via a ones-row in h2
    woutt = nc.dram_tensor("woutt", [H2 + 1, 1], f32, kind="ExternalInput")
    out = nc.dram_tensor("out", [NCH, CHUNK], f32, kind="ExternalOutput")

    with tile.TileContext(nc) as tc:
        with ExitStack() as ctx:
            const = ctx.enter_context(tc.tile_pool(name="const", bufs=1))
            xfp = ctx.enter_context(tc.tile_pool(name="xf", bufs=5))
            xbp = ctx.enter_context(tc.tile_pool(name="xb", bufs=3))
            drp = ctx.enter_context(tc.tile_pool(name="drain", bufs=4))
            h1rp = ctx.enter_context(tc.tile_pool(name="h1r", bufs=4))
            h1ap = ctx.enter_context(tc.tile_pool(name="h1a", bufs=4))
            smp = ctx.enter_context(tc.tile_pool(name="small", bufs=4))
            ps1 = ctx.enter_context(tc.tile_pool(name="ps1", bufs=2, space="PSUM"))
            ps2 = ctx.enter_context(tc.tile_pool(name="ps2", bufs=2, space="PSUM"))
            ps3 = ctx.enter_context(tc.tile_pool(name="ps3", bufs=2, space="PSUM"))
            dram = ctx.enter_context(tc.tile_pool(name="dram", bufs=2 * NCH, space="DRAM"))

            # ---- resident constants ----
            # W1 hi/lo stay resident all kernel, but are DMA'd in NSUP slices
            # interleaved with chunk 0's x loads so the PE can start without
            # waiting for the full 12.6 MB.
            w1hi_sl = [
                const.tile([P, SUP, H1], bf16, name=f"w1hi_sl{s}", tag=f"w1hi{s}")
                for s in range(NSUP)
            ]
            w1lo_sl = [
                const.tile([P, SUP, H1], bf16, name=f"w1lo_sl{s}", tag=f"w1lo{s}")
                for s in range(NSUP)
            ] if W1_PASSES == 2 else None
            b1_s = const.tile([P, M_T], f32)
            nc.sync.dma_start(b1_s[:], b1.ap())
            w2t_s = const.tile([P, M_T, H2], f32)
            nc.sync.dma_start(w2t_s[:], w2t.ap())
            b2_s = const.tile([H2, 1], f32)
            nc.sync.dma_start(b2_s[:], b2.ap())
            woutt_s = const.tile([H2 + 1, 1], f32)
            nc.sync.dma_start(woutt_s[:], woutt.ap())

            # ---- phase 2: bias+relu, fc2, fc3 for one chunk ----
            def phase2(j):
                cc_out = cc_outs[j]
                h1a = []
                for m in range(M_T):
                    h1r = h1rp.tile([P, CHUNK], f32, name=f"h1r{j}_{m}", tag="h1r")
                    nc.sync.dma_start(h1r[:], cc_out[m * P:(m + 1) * P, :])
                    act = h1ap.tile([P, CHUNK], f32, name=f"act{j}_{m}", tag="act")
                    nc.scalar.activation(
                        act[:], h1r[:],
                        mybir.ActivationFunctionType.Relu,
                        bias=b1_s[:, m:m + 1],
                    )
                    h1a.append(act)

                p2 = ps2.tile([H2, CHUNK], f32, name=f"p2_{j}", tag="p2")
                for m in range(M_T):
                    nc.tensor.matmul(
                        p2[:], w2t_s[:, m, :], h1a[m][:],
                        start=(m == 0), stop=(m == M_T - 1),
                    )
                h2t = smp.tile([H2 + 1, CHUNK], f32, tag="h2", name=f"h2t{j}")
                nc.scalar.activation(
                    h2t[0:H2, :], p2[:],
                    mybir.ActivationFunctionType.Relu,
                    bias=b2_s[:],
                )
                nc.vector.memset(h2t[H2:H2 + 1, :], 1.0)

                p3 = ps3.tile([1, CHUNK], f32, name=f"p3_{j}", tag="p3")
                nc.tensor.matmul(p3[:], woutt_s[:], h2t[:], start=True, stop=True)
                ot = smp.tile([1, CHUNK], f32, tag="ot", name=f"ot{j}")
                nc.vector.tensor_copy(ot[:], p3[:])
                nc.sync.dma_start(out.ap()[j, :], ot[:])

            # ---- phase 1: fc1 for all chunks; each chunk's partial goes
            # straight into its AllReduce so the collectives overlap the
            # next chunk's matmuls. Chunk j-1's fc2/fc3 are emitted after
            # chunk j's fc1 so the PE never idle-waits on an AllReduce
            # except for the final chunk's.
            cc_outs = []
            _loaded_w1 = set()
            for j in range(NCH):
                psum_m = [
                    ps1.tile([P, CHUNK], f32, tag=f"ps1_{m}", name=f"ps1m{m}_c{j}")
                    for m in range(M_T)
                ]
                # first super-tile of the kernel is small so the PE starts early
                sched = [2, 6] + [SUP] * (NSUP - 1) if j == 0 else [SUP] * NSUP
                s_off = 0  # k-tile offset
                for si, sup in enumerate(sched):
                    if j == 0:
                        # W1 slices are aligned to the fixed SUP grid
                        for s in range(s_off // SUP, (s_off + sup + SUP - 1) // SUP):
                            if s < NSUP and s not in _loaded_w1:
                                _loaded_w1.add(s)
                                nc.sync.dma_start(
                                    w1hi_sl[s][:], w1hi.ap()[:, s * SUP:(s + 1) * SUP, :]
                                )
                                if W1_PASSES == 2:
                                    nc.sync.dma_start(
                                        w1lo_sl[s][:], w1lo.ap()[:, s * SUP:(s + 1) * SUP, :]
                                    )
                    xf = xfp.tile([P, SUP, CHUNK], f32, name=f"xf_{j}_{si}", tag="xf")
                    nc.sync.dma_start(
                        xf[:, 0:sup, :],
                        xt.ap()[j, :, :, :, :].rearrange("p s t n -> p (s t) n")[
                            :, s_off:s_off + sup, :
                        ],
                    )
                    xb = xbp.tile([P, SUP, CHUNK], bf16, name=f"xb_{j}_{si}", tag="xb")
                    nc.vector.tensor_copy(xb[:, 0:sup, :], xf[:, 0:sup, :])
                    for tt in range(sup):
                        t = s_off + tt
                        s, ts = divmod(t, SUP)
                        for m in range(M_T):
                            nc.tensor.matmul(
                                psum_m[m][:],
                                w1hi_sl[s][:, ts, m * P:(m + 1) * P],
                                xb[:, tt, :],
                                start=(t == 0),
                                stop=(W1_PASSES == 1 and t == KT - 1),
                            )
                            if W1_PASSES == 2:
                                nc.tensor.matmul(
                                    psum_m[m][:],
                                    w1lo_sl[s][:, ts, m * P:(m + 1) * P],
                                    xb[:, tt, :],
                                    start=False,
                                    stop=(t == KT - 1),
                                )
                    s_off += sup

                cc_in = dram.tile([H1, CHUNK], f32, tag="cc_in", name=f"cc_in{j}")
                cc_out = dram.tile([H1, CHUNK], f32, tag="cc_out", name=f"cc_out{j}")
                for m in range(M_T):
                    dr = drp.tile([P, CHUNK], f32, name=f"dr{j}_{m}", tag="dr")
                    nc.vector.tensor_copy(dr[:], psum_m[m][:])
                    nc.sync.dma_start(cc_in[m * P:(m + 1) * P, :], dr[:])
                nc.gpsimd.collective_compute(
                    "AllReduce",
                    mybir.AluOpType.add,
                    replica_groups=[list(range(N_CORES))],
                    ins=[cc_in.opt()],
                    outs=[cc_out.opt()],
                )
                cc_outs.append(cc_out)
                if j > 0:
                    phase2(j - 1)
            phase2(NCH - 1)

    nc.compile()
    return nc


def get_program():
    if "nc" not in _CACHED:
        _CACHED["nc"] = _build_program()
    return _CACHED["nc"]


def _prep_inputs(x, W1, b1, W2, b2, Wout, bout):
    """Shard + lay out host-side into DMA-friendly per-core tensors."""
    bf = ml_dtypes.bfloat16

    # x: [2048, 98304] -> xT [98304, 2048] -> per core [NCH, P, NSUP, SUP, CHUNK]
    xT = np.ascontiguousarray(x.T)  # [98304, 2048]
    # x6[c] axes: [s, t, p, j, n]; device wants [j, p, s, t, n]
    x6 = xT.reshape(N_CORES, NSUP, SUP, P, NCH, CHUNK)
    in_maps = [
        {"xt": np.ascontiguousarray(x6[c].transpose(3, 2, 0, 1, 4))}
        for c in range(N_CORES)
    ]

    w1T = np.ascontiguousarray(W1.T)  # [98304, 256]
    b1_h = np.ascontiguousarray(b1.reshape(M_T, P).T)            # [P, M_T]
    w2t_h = np.ascontiguousarray(W2.T.reshape(M_T, P, H2).transpose(1, 0, 2))  # [P, M_T, H2]
    b2_h = np.ascontiguousarray(b2.reshape(H2, 1))
    woutt_h = np.concatenate(
        [Wout.T, bout.reshape(1, 1)], axis=0
    ).astype(np.float32)                                         # [H2+1, 1]

    for c in range(N_CORES):
        w1T_c = w1T[c * KSH:(c + 1) * KSH]                       # [12288, 256]
        hi = w1T_c.astype(bf)
        # [KSH, H1] -> [P, KT, H1]: row (t*P + p) -> [p, t]
        hi = np.ascontiguousarray(hi.reshape(KT, P, H1).transpose(1, 0, 2))
        in_maps[c].update({
            "w1hi": hi,
            "b1": b1_h,
            "w2t": w2t_h,
            "b2": b2_h,
            "woutt": woutt_h,
        })
        if W1_PASSES == 2:
            lo = (w1T_c - in_maps[c]["w1hi"].astype(np.float32).transpose(1, 0, 2).reshape(KSH, H1)).astype(bf)
            in_maps[c]["w1lo"] = np.ascontiguousarray(lo.reshape(KT, P, H1).transpose(1, 0, 2))
    return in_maps


def kernel(x, W1, b1, W2, b2, Wout, bout, _trace=False, _trace_kwargs=None):
    x = np.asarray(x, dtype=np.float32)
    W1 = np.asarray(W1, dtype=np.float32)
    b1 = np.asarray(b1, dtype=np.float32)
    W2 = np.asarray(W2, dtype=np.float32)
    b2 = np.asarray(b2, dtype=np.float32)
    Wout = np.asarray(Wout, dtype=np.float32)
    bout = np.asarray(bout, dtype=np.float32)

    nc = get_program()
    in_maps = _prep_inputs(x, W1, b1, W2, b2, Wout, bout)
    res = run_bass_kernel_spmd(
        nc,
        in_maps,
        core_ids=list(range(N_CORES)),
        trace=_trace,
        **(_trace_kwargs or {}),
    )
    out = res.results[0]["out"].reshape(B, 1).astype(np.float32)
    if _trace:
        kernel.last_results = res
    return out


if __name__ == "__main__":
    # quick self-run with random data (not the reference distribution)
    rng = np.random.default_rng(0)
    x = (rng.random((B, IN_DIM)) < 32.0 / IN_DIM).astype(np.float32)
    W1 = rng.standard_normal((H1, IN_DIM), dtype=np.float32) / np.sqrt(IN_DIM)
    b1 = rng.standard_normal(H1, dtype=np.float32) / np.sqrt(IN_DIM)
    W2 = rng.standard_normal((H2, H1), dtype=np.float32) / np.sqrt(H1)
    b2 = rng.standard_normal(H2, dtype=np.float32) / np.sqrt(H1)
    Wout = rng.standard_normal((1, H2), dtype=np.float32) / np.sqrt(H2)
    bout = rng.standard_normal(1, dtype=np.float32) / np.sqrt(H2)
    got = kernel(x, W1, b1, W2, b2, Wout, bout)
    h1 = np.maximum(x @ W1.T + b1, 0)
    h2 = np.maximum(h1 @ W2.T + b2, 0)
    exp = h2 @ Wout.T + bout
    print("rel err:", np.abs(got - exp).max() / np.abs(exp).max())



# revision 9
# speedup vs baseline: 3.0145x; 3.0145x over previous
"""HalfKA NNUE forward pass on 8 Trainium2 NeuronCores — sparse gather version.

Network (fp32 reference):
    h1  = relu(x @ W1.T + b1)     x:[2048, 98304] sparse 0/1, W1:[256, 98304]
    h2  = relu(h1 @ W2.T + b2)    W2:[32, 256]
    out = h2 @ Wout.T + bout      Wout:[1, 32]  -> [2048, 1]

x is a few-hot mask (~32 active features per row), so fc1 is an embedding
lookup: h1[b] = sum_{i in active(b)} W1[:, i] + b1. Instead of streaming the
805 MB dense x, the host converts each row to its active-index list and the
device gathers the corresponding 256-dim bf16 embedding columns straight from
HBM with dma_gather (~0.5 MB/core of random 512 B reads).

Sharding: data-parallel over batch — core c owns rows [256c, 256(c+1)), no
collectives. The bf16 embedding table (W1.T) is replicated in every core's
DRAM, split into 4 chunks of 24576 rows (+1 zero pad row each) because
dma_gather indices are int16.

Per core, slots are grouped per (chunk, band-of-128-rows) and padded to a
fixed 1280 (actual max 1116) with zero-row pads. The gathered block
G[slot, emb] for each 128-slot group is reduced into per-row h1 on the PE:
      h1T[emb, row] += G[:, emb].T @ S[:, row]
where S[slot, row] = (rowid[slot] == row) is a one-hot selection matrix built
on the DVE from host-shipped row ids (pads get rowid -1 => zero column). The
result lands directly in the [emb-partition, batch-free] layout that fc2
wants, so bias+relu is a single activation per psum tile and fc2/fc3 are the
same tiny matmuls as the dense kernel.
"""

import sys

sys.path.insert(0, "/opt/trn_rl_repo")

from contextlib import ExitStack

import numpy as np
import ml_dtypes

import concourse.bass as bass  # noqa: F401  (registers engine libraries)
import concourse.tile as tile
from concourse import bacc, mybir
from concourse.bass_utils import run_bass_kernel_spmd

f32 = mybir.dt.float32
bf16 = mybir.dt.bfloat16
i16 = mybir.dt.int16

N_CORES = 8
B = 2048
IN_DIM = 98304
H1 = 256
H2 = 32

RPC = B // N_CORES      # 256 rows per core
BANDS = 2               # 128-row PE bands per core
NCH = 4                 # embedding-table chunks (int16 index range)
CHR = IN_DIM // NCH     # 24576 feature rows per chunk
ZROW = CHR              # zero row appended at the end of each chunk
NPB = 1280              # padded slots per (chunk, band); actual max 1116
GPB = NPB // 128        # 10 groups of 128 slots per band
NPC = NPB * BANDS       # 2560 slots per chunk-gather
GPC = GPB * BANDS       # 20 groups per chunk
MH = H1 // 128          # 2 psum halves of the 256-dim h1

_CACHED = {}


def _build_program():
    nc = bacc.Bacc(
        "TRN2",
        target_bir_lowering=False,
        debug=False,
        num_devices=N_CORES,
    )

    table = nc.dram_tensor("table", [NCH, CHR + 1, H1], bf16, kind="ExternalInput")
    idx_d = nc.dram_tensor(
        "idx", [128, NCH, BANDS, NPB // 16], i16, kind="ExternalInput"
    )
    rid_d = nc.dram_tensor("rid", [128, NCH, GPC], bf16, kind="ExternalInput")
    iota_d = nc.dram_tensor("iota", [128, 128], bf16, kind="ExternalInput")
    b1_d = nc.dram_tensor("b1", [128, MH], f32, kind="ExternalInput")
    w2t_d = nc.dram_tensor("w2t", [128, MH, H2], f32, kind="ExternalInput")
    b2_d = nc.dram_tensor("b2", [H2, 1], f32, kind="ExternalInput")
    wout_d = nc.dram_tensor("woutt", [H2 + 1, 1], f32, kind="ExternalInput")
    out = nc.dram_tensor("out", [RPC], f32, kind="ExternalOutput")

    with tile.TileContext(nc) as tc:
        with ExitStack() as ctx:
            const = ctx.enter_context(tc.tile_pool(name="const", bufs=1))
            gp = ctx.enter_context(tc.tile_pool(name="g", bufs=1))
            sp = ctx.enter_context(tc.tile_pool(name="s", bufs=1))
            smp = ctx.enter_context(tc.tile_pool(name="small", bufs=1))
            psa = ctx.enter_context(
                tc.tile_pool(name="psa", bufs=1, space="PSUM")
            )
            ps2 = ctx.enter_context(tc.tile_pool(name="ps2", bufs=1, space="PSUM"))
            ps3 = ctx.enter_context(tc.tile_pool(name="ps3", bufs=1, space="PSUM"))

            idxt = const.tile([128, NCH, BANDS, NPB // 16], i16)
            nc.sync.dma_start(idxt[:], idx_d.ap())
            ridt = const.tile([128, NCH, GPC], bf16)
            nc.sync.dma_start(ridt[:], rid_d.ap())
            iota = const.tile([128, 128], bf16)
            nc.sync.dma_start(iota[:], iota_d.ap())
            b1_s = const.tile([128, MH], f32)
            nc.sync.dma_start(b1_s[:], b1_d.ap())
            w2t_s = const.tile([128, MH, H2], f32)
            nc.sync.dma_start(w2t_s[:], w2t_d.ap())
            b2_s = const.tile([H2, 1], f32)
            nc.sync.dma_start(b2_s[:], b2_d.ap())
            wout_s = const.tile([H2 + 1, 1], f32)
            nc.sync.dma_start(wout_s[:], wout_d.ap())

            # gathers per (table chunk, band), each split in two 640-idx
            # instructions: >1024 idxs in one dma_gather overflows the SWDGE
            # descriptor-ring carveout and deadlocks the ucode's await_space
            # on hardware. slot i -> gt[i % 128, i // 128, :]
            HG = GPB // 2          # groups per gather instruction (5)
            NHG = HG * 128         # idxs per gather instruction (640)
            gts = {}
            for c in range(NCH):
                for b in range(BANDS):
                    gt = gp.tile([128, GPB, H1], bf16, name=f"g{c}_{b}",
                                 tag=f"g{c}_{b}")
                    for v in range(2):
                        nc.gpsimd.dma_gather(
                            gt[:, v * HG:(v + 1) * HG, :],
                            table.ap()[c],
                            idxt[:, c, b, v * (NHG // 16):(v + 1) * (NHG // 16)],
                            NHG,
                            NHG,
                            H1,
                        )
                    gts[c, b] = gt

            # selection matrices S[c,b][slot, g, row] = (rowid == row)
            sts = {}
            for c in range(NCH):
                for b in range(BANDS):
                    st = sp.tile([128, GPB, 128], bf16, name=f"s{c}_{b}", tag=f"s{c}_{b}")
                    nc.vector.scalar_tensor_tensor(
                        st[:],
                        ridt[:, c, b * GPB:(b + 1) * GPB]
                        .unsqueeze(2)
                        .broadcast_to([128, GPB, 128]),
                        0.0,
                        iota[:].unsqueeze(1).broadcast_to([128, GPB, 128]),
                        mybir.AluOpType.add,
                        mybir.AluOpType.is_equal,
                    )
                    sts[c, b] = st

            # fc1: psum[b][h][emb, row] += G[slot, emb].T @ S[slot, row]
            psum = [
                [psa.tile([128, 128], f32, name=f"ps{b}_{h}") for h in range(MH)]
                for b in range(BANDS)
            ]
            for c in range(NCH):
                for b in range(BANDS):
                    for g in range(GPB):
                        for h in range(MH):
                            nc.tensor.matmul(
                                psum[b][h][:],
                                gts[c, b][:, g, h * 128:(h + 1) * 128],
                                sts[c, b][:, g, :],
                                start=(c == 0 and g == 0),
                                stop=(c == NCH - 1 and g == GPB - 1),
                            )

            # h1 = relu(fc1 + b1), already transposed: [emb-part, MH, row]
            h1t = smp.tile([128, MH, RPC], f32, name="h1t")
            for b in range(BANDS):
                for h in range(MH):
                    nc.scalar.activation(
                        h1t[:, h, b * 128:(b + 1) * 128],
                        psum[b][h][:],
                        mybir.ActivationFunctionType.Relu,
                        bias=b1_s[:, h:h + 1],
                    )

            # fc2
            p2 = ps2.tile([H2, RPC], f32, name="p2")
            for h in range(MH):
                nc.tensor.matmul(
                    p2[:], w2t_s[:, h, :], h1t[:, h, :],
                    start=(h == 0), stop=(h == MH - 1),
                )
            h2t = smp.tile([H2 + 1, RPC], f32, name="h2t")
            nc.scalar.activation(
                h2t[0:H2, :], p2[:],
                mybir.ActivationFunctionType.Relu,
                bias=b2_s[:],
            )
            nc.vector.memset(h2t[H2:H2 + 1, :], 1.0)

            # fc3 (bout folded in via the ones row)
            p3 = ps3.tile([1, RPC], f32, name="p3")
            nc.tensor.matmul(p3[:], wout_s[:], h2t[:], start=True, stop=True)
            ot = smp.tile([1, RPC], f32, name="ot")
            nc.vector.tensor_copy(ot[:], p3[:])
            nc.sync.dma_start(out.ap(), ot[:])

    nc.compile()
    return nc


def get_program():
    if "nc" not in _CACHED:
        _CACHED["nc"] = _build_program()
    return _CACHED["nc"]


def _prep_inputs(x, W1, b1, W2, b2, Wout, bout):
    """Convert the dense few-hot x into per-core gather index lists and build
    the shared bf16 embedding table + small fc weights."""
    bf = ml_dtypes.bfloat16

    w1T = np.ascontiguousarray(W1.T).astype(bf)             # [IN_DIM, H1]
    table = np.zeros((NCH, CHR + 1, H1), dtype=bf)
    table[:, :CHR, :] = w1T.reshape(NCH, CHR, H1)

    b1_h = np.ascontiguousarray(b1.reshape(MH, 128).T)      # [128, MH]
    w2t_h = np.ascontiguousarray(
        W2.T.reshape(MH, 128, H2).transpose(1, 0, 2)        # [128, MH, H2]
    )
    b2_h = np.ascontiguousarray(b2.reshape(H2, 1)).astype(np.float32)
    wout_h = np.concatenate(
        [Wout.T, bout.reshape(1, 1)], axis=0
    ).astype(np.float32)                                    # [H2+1, 1]
    iota_h = np.ascontiguousarray(
        np.broadcast_to(np.arange(128, dtype=np.float32), (128, 128))
    ).astype(bf)

    rows, cols = np.nonzero(x)                              # row-major sorted
    in_maps = []
    for cidx in range(N_CORES):
        m = (rows >= cidx * RPC) & (rows < (cidx + 1) * RPC)
        r = rows[m] - cidx * RPC
        f = cols[m]
        ch = f // CHR
        band = r // 128

        idx_arr = np.full((NCH, NPC), ZROW, dtype=np.int16)
        rid_arr = np.full((NCH, GPC, 128), -1.0, dtype=np.float32)
        for c in range(NCH):
            for b in range(BANDS):
                sel = (ch == c) & (band == b)
                n = int(sel.sum())
                assert n <= NPB, f"slot padding overflow: {n} > {NPB}"
                pos = b * NPB + np.arange(n)
                idx_arr[c, pos] = (f[sel] - c * CHR).astype(np.int16)
                rid_arr[c, pos // 128, pos % 128] = r[sel] - b * 128

        # dma_gather reads slot i's index at idxs[i % 16, i // 16], replicated
        # across the eight 16-partition gpsimd cores
        w = idx_arr.reshape(NCH, BANDS, NPB // 16, 16)      # [c, b, s, j]
        idx_t = np.ascontiguousarray(
            np.tile(w.transpose(3, 0, 1, 2), (8, 1, 1, 1))  # [128, c, b, s]
        )
        rid_t = np.ascontiguousarray(
            rid_arr.transpose(2, 0, 1).astype(bf)           # [128, NCH, GPC]
        )
        in_maps.append({
            "table": table,
            "idx": idx_t,
            "rid": rid_t,
            "iota": iota_h,
            "b1": b1_h,
            "w2t": w2t_h,
            "b2": b2_h,
            "woutt": wout_h,
        })
    return in_maps


def kernel(x, W1, b1, W2, b2, Wout, bout, _trace=False, _trace_kwargs=None):
    x = np.asarray(x, dtype=np.float32)
    W1 = np.asarray(W1, dtype=np.float32)
    b1 = np.asarray(b1, dtype=np.float32)
    W2 = np.asarray(W2, dtype=np.float32)
    b2 = np.asarray(b2, dtype=np.float32)
    Wout = np.asarray(Wout, dtype=np.float32)
    bout = np.asarray(bout, dtype=np.float32)

    nc = get_program()
    in_maps = _prep_inputs(x, W1, b1, W2, b2, Wout, bout)
    res = run_bass_kernel_spmd(
        nc,
        in_maps,
        core_ids=list(range(N_CORES)),
        trace=_trace,
        **(_trace_kwargs or {}),
    )
    out = np.concatenate(
        [res.results[c]["out"] for c in range(N_CORES)]
    ).reshape(B, 1).astype(np.float32)
    if _trace:
        kernel.last_results = res
    return out


if __name__ == "__main__":
    # quick self-run with random data (not the reference distribution)
    rng = np.random.default_rng(0)
    x = (rng.random((B, IN_DIM)) < 32.0 / IN_DIM).astype(np.float32)
    W1 = rng.standard_normal((H1, IN_DIM), dtype=np.float32) / np.sqrt(IN_DIM)
    b1 = rng.standard_normal(H1, dtype=np.float32) / np.sqrt(IN_DIM)
    W2 = rng.standard_normal((H2, H1), dtype=np.float32) / np.sqrt(H1)
    b2 = rng.standard_normal(H2, dtype=np.float32) / np.sqrt(H1)
    Wout = rng.standard_normal((1, H2), dtype=np.float32) / np.sqrt(H2)
    bout = rng.standard_normal(1, dtype=np.float32) / np.sqrt(H2)
    got = kernel(x, W1, b1, W2, b2, Wout, bout)
    h1 = np.maximum(x @ W1.T + b1, 0)
    h2 = np.maximum(h1 @ W2.T + b2, 0)
    exp = h2 @ Wout.T + bout
    print("rel err:", np.abs(got - exp).max() / np.abs(exp).max())


# revision 12
# speedup vs baseline: 5.9197x; 1.9637x over previous
"""HalfKA NNUE forward pass on 8 Trainium2 NeuronCores — sparse gather version.

Network (fp32 reference):
    h1  = relu(x @ W1.T + b1)     x:[2048, 98304] sparse 0/1, W1:[256, 98304]
    h2  = relu(h1 @ W2.T + b2)    W2:[32, 256]
    out = h2 @ Wout.T + bout      Wout:[1, 32]  -> [2048, 1]

x is a few-hot mask (~32 active features per row), so fc1 is an embedding
lookup: h1[b] = sum_{i in active(b)} W1[:, i] + b1. Instead of streaming the
805 MB dense x, the host converts each row to its active-index list and the
device gathers the corresponding 256-dim bf16 embedding columns straight from
HBM with dma_gather (~0.5 MB/core of random 512 B reads).

Sharding: data-parallel over batch — core c owns rows [256c, 256(c+1)), no
collectives. The bf16 embedding table (W1.T) is replicated in every core's
DRAM, split into 4 chunks of 24576 rows (+1 zero pad row each) because
dma_gather indices are int16.

Per core, slots are grouped per (chunk, band-of-128-rows) and padded to a
fixed 1280 (actual max 1116) with zero-row pads. The gathered block
G[slot, emb] for each 128-slot group is reduced into per-row h1 on the PE:
      h1T[emb, row] += G[:, emb].T @ S[:, row]
where S[slot, row] = (rowid[slot] == row) is a one-hot selection matrix built
on the DVE from host-shipped row ids (pads get rowid -1 => zero column). The
result lands directly in the [emb-partition, batch-free] layout that fc2
wants, so bias+relu is a single activation per psum tile and fc2/fc3 are the
same tiny matmuls as the dense kernel.
"""

import sys

sys.path.insert(0, "/opt/trn_rl_repo")

from contextlib import ExitStack

import numpy as np
import ml_dtypes

import concourse.bass as bass  # noqa: F401  (registers engine libraries)
import concourse.tile as tile
from concourse import bacc, mybir
from concourse.bass_utils import run_bass_kernel_spmd

f32 = mybir.dt.float32
bf16 = mybir.dt.bfloat16
i16 = mybir.dt.int16

N_CORES = 8
B = 2048
IN_DIM = 98304
H1 = 256
H2 = 32

RPC = B // N_CORES      # 256 rows per core
BANDS = 2               # 128-row PE bands per core
NCH = 4                 # embedding-table chunks (int16 index range)
CHR = IN_DIM // NCH     # 24576 feature rows per chunk
ZROW = CHR              # zero row appended at the end of each chunk
NPB = 1280              # padded slots per (chunk, band); actual max 1116
GPB = NPB // 128        # 10 groups of 128 slots per band
NPC = NPB * BANDS       # 2560 slots per chunk-gather
GPC = GPB * BANDS       # 20 groups per chunk
MH = H1 // 128          # 2 psum halves of the 256-dim h1

_CACHED = {}


def _build_program():
    nc = bacc.Bacc(
        "TRN2",
        target_bir_lowering=False,
        debug=False,
        num_devices=N_CORES,
        num_swdge_queues=4,
    )

    table = nc.dram_tensor("table", [NCH, CHR + 1, H1], bf16, kind="ExternalInput")
    idx_d = nc.dram_tensor(
        "idx", [128, NCH, BANDS, NPB // 16], i16, kind="ExternalInput"
    )
    rid_d = nc.dram_tensor("rid", [128, NCH, GPC], bf16, kind="ExternalInput")
    iota_d = nc.dram_tensor("iota", [128, 128], bf16, kind="ExternalInput")
    b1_d = nc.dram_tensor("b1", [128, MH], f32, kind="ExternalInput")
    w2t_d = nc.dram_tensor("w2t", [128, MH, H2], f32, kind="ExternalInput")
    b2_d = nc.dram_tensor("b2", [H2, 1], f32, kind="ExternalInput")
    wout_d = nc.dram_tensor("woutt", [H2 + 1, 1], f32, kind="ExternalInput")
    out = nc.dram_tensor("out", [RPC], f32, kind="ExternalOutput")

    with tile.TileContext(nc) as tc:
        with ExitStack() as ctx:
            const = ctx.enter_context(tc.tile_pool(name="const", bufs=1))
            gp = ctx.enter_context(tc.tile_pool(name="g", bufs=1))
            sp = ctx.enter_context(tc.tile_pool(name="s", bufs=1))
            smp = ctx.enter_context(tc.tile_pool(name="small", bufs=1))
            psa = ctx.enter_context(
                tc.tile_pool(name="psa", bufs=1, space="PSUM")
            )
            ps2 = ctx.enter_context(tc.tile_pool(name="ps2", bufs=1, space="PSUM"))
            ps3 = ctx.enter_context(tc.tile_pool(name="ps3", bufs=1, space="PSUM"))

            idxt = const.tile([128, NCH, BANDS, NPB // 16], i16)
            nc.sync.dma_start(idxt[:], idx_d.ap())
            ridt = const.tile([128, NCH, GPC], bf16)
            nc.sync.dma_start(ridt[:], rid_d.ap())
            iota = const.tile([128, 128], bf16)
            nc.sync.dma_start(iota[:], iota_d.ap())
            b1_s = const.tile([128, MH], f32)
            nc.sync.dma_start(b1_s[:], b1_d.ap())
            w2t_s = const.tile([128, MH, H2], f32)
            nc.sync.dma_start(w2t_s[:], w2t_d.ap())
            b2_s = const.tile([H2, 1], f32)
            nc.sync.dma_start(b2_s[:], b2_d.ap())
            wout_s = const.tile([H2 + 1, 1], f32)
            nc.sync.dma_start(wout_s[:], wout_d.ap())

            # gathers per (table chunk, band), each split in two 640-idx
            # instructions: >1024 idxs in one dma_gather overflows the SWDGE
            # descriptor-ring carveout and deadlocks the ucode's await_space
            # on hardware. slot i -> gt[i % 128, i // 128, :]
            HG = GPB // 2          # groups per gather instruction (5)
            NHG = HG * 128         # idxs per gather instruction (640)
            gts = {}
            nq = 0
            for c in range(NCH):
                for b in range(BANDS):
                    gt = gp.tile([128, GPB, H1], bf16, name=f"g{c}_{b}",
                                 tag=f"g{c}_{b}")
                    for v in range(2):
                        nc.gpsimd.dma_gather(
                            gt[:, v * HG:(v + 1) * HG, :],
                            table.ap()[c],
                            idxt[:, c, b, v * (NHG // 16):(v + 1) * (NHG // 16)],
                            NHG,
                            NHG,
                            H1,
                            queue_num=nq % 4,
                        )
                        nq += 1
                    gts[c, b] = gt

            # selection matrices S[c,b][slot, g, row] = (rowid == row)
            sts = {}
            for c in range(NCH):
                for b in range(BANDS):
                    st = sp.tile([128, GPB, 128], bf16, name=f"s{c}_{b}", tag=f"s{c}_{b}")
                    nc.vector.scalar_tensor_tensor(
                        st[:],
                        ridt[:, c, b * GPB:(b + 1) * GPB]
                        .unsqueeze(2)
                        .broadcast_to([128, GPB, 128]),
                        0.0,
                        iota[:].unsqueeze(1).broadcast_to([128, GPB, 128]),
                        mybir.AluOpType.add,
                        mybir.AluOpType.is_equal,
                    )
                    sts[c, b] = st

            # fc1: psum[b][h][emb, row] += G[slot, emb].T @ S[slot, row]
            psum = [
                [psa.tile([128, 128], f32, name=f"ps{b}_{h}") for h in range(MH)]
                for b in range(BANDS)
            ]
            for c in range(NCH):
                for b in range(BANDS):
                    for g in range(GPB):
                        for h in range(MH):
                            nc.tensor.matmul(
                                psum[b][h][:],
                                gts[c, b][:, g, h * 128:(h + 1) * 128],
                                sts[c, b][:, g, :],
                                start=(c == 0 and g == 0),
                                stop=(c == NCH - 1 and g == GPB - 1),
                            )

            # h1 = relu(fc1 + b1), already transposed: [emb-part, MH, row]
            h1t = smp.tile([128, MH, RPC], f32, name="h1t")
            for b in range(BANDS):
                for h in range(MH):
                    nc.scalar.activation(
                        h1t[:, h, b * 128:(b + 1) * 128],
                        psum[b][h][:],
                        mybir.ActivationFunctionType.Relu,
                        bias=b1_s[:, h:h + 1],
                    )

            # fc2
            p2 = ps2.tile([H2, RPC], f32, name="p2")
            for h in range(MH):
                nc.tensor.matmul(
                    p2[:], w2t_s[:, h, :], h1t[:, h, :],
                    start=(h == 0), stop=(h == MH - 1),
                )
            h2t = smp.tile([H2 + 1, RPC], f32, name="h2t")
            nc.scalar.activation(
                h2t[0:H2, :], p2[:],
                mybir.ActivationFunctionType.Relu,
                bias=b2_s[:],
            )
            nc.vector.memset(h2t[H2:H2 + 1, :], 1.0)

            # fc3 (bout folded in via the ones row)
            p3 = ps3.tile([1, RPC], f32, name="p3")
            nc.tensor.matmul(p3[:], wout_s[:], h2t[:], start=True, stop=True)
            ot = smp.tile([1, RPC], f32, name="ot")
            nc.vector.tensor_copy(ot[:], p3[:])
            nc.sync.dma_start(out.ap(), ot[:])

    nc.compile()
    return nc


def get_program():
    if "nc" not in _CACHED:
        _CACHED["nc"] = _build_program()
    return _CACHED["nc"]


def _prep_inputs(x, W1, b1, W2, b2, Wout, bout):
    """Convert the dense few-hot x into per-core gather index lists and build
    the shared bf16 embedding table + small fc weights."""
    bf = ml_dtypes.bfloat16

    w1T = np.ascontiguousarray(W1.T).astype(bf)             # [IN_DIM, H1]
    table = np.zeros((NCH, CHR + 1, H1), dtype=bf)
    table[:, :CHR, :] = w1T.reshape(NCH, CHR, H1)

    b1_h = np.ascontiguousarray(b1.reshape(MH, 128).T)      # [128, MH]
    w2t_h = np.ascontiguousarray(
        W2.T.reshape(MH, 128, H2).transpose(1, 0, 2)        # [128, MH, H2]
    )
    b2_h = np.ascontiguousarray(b2.reshape(H2, 1)).astype(np.float32)
    wout_h = np.concatenate(
        [Wout.T, bout.reshape(1, 1)], axis=0
    ).astype(np.float32)                                    # [H2+1, 1]
    iota_h = np.ascontiguousarray(
        np.broadcast_to(np.arange(128, dtype=np.float32), (128, 128))
    ).astype(bf)

    rows, cols = np.nonzero(x)                              # row-major sorted
    in_maps = []
    for cidx in range(N_CORES):
        m = (rows >= cidx * RPC) & (rows < (cidx + 1) * RPC)
        r = rows[m] - cidx * RPC
        f = cols[m]
        ch = f // CHR
        band = r // 128

        idx_arr = np.full((NCH, NPC), ZROW, dtype=np.int16)
        rid_arr = np.full((NCH, GPC, 128), -1.0, dtype=np.float32)
        for c in range(NCH):
            for b in range(BANDS):
                sel = (ch == c) & (band == b)
                n = int(sel.sum())
                assert n <= NPB, f"slot padding overflow: {n} > {NPB}"
                # ascending feature order -> DMA descriptors walk increasing
                # HBM addresses (S reassigns slots to rows, any order works)
                order = np.argsort(f[sel], kind="stable")
                pos = b * NPB + np.arange(n)
                idx_arr[c, pos] = (f[sel][order] - c * CHR).astype(np.int16)
                rid_arr[c, pos // 128, pos % 128] = r[sel][order] - b * 128

        # dma_gather reads slot i's index at idxs[i % 16, i // 16], replicated
        # across the eight 16-partition gpsimd cores
        w = idx_arr.reshape(NCH, BANDS, NPB // 16, 16)      # [c, b, s, j]
        idx_t = np.ascontiguousarray(
            np.tile(w.transpose(3, 0, 1, 2), (8, 1, 1, 1))  # [128, c, b, s]
        )
        rid_t = np.ascontiguousarray(
            rid_arr.transpose(2, 0, 1).astype(bf)           # [128, NCH, GPC]
        )
        in_maps.append({
            "table": table,
            "idx": idx_t,
            "rid": rid_t,
            "iota": iota_h,
            "b1": b1_h,
            "w2t": w2t_h,
            "b2": b2_h,
            "woutt": wout_h,
        })
    return in_maps


def kernel(x, W1, b1, W2, b2, Wout, bout, _trace=False, _trace_kwargs=None):
    x = np.asarray(x, dtype=np.float32)
    W1 = np.asarray(W1, dtype=np.float32)
    b1 = np.asarray(b1, dtype=np.float32)
    W2 = np.asarray(W2, dtype=np.float32)
    b2 = np.asarray(b2, dtype=np.float32)
    Wout = np.asarray(Wout, dtype=np.float32)
    bout = np.asarray(bout, dtype=np.float32)

    nc = get_program()
    in_maps = _prep_inputs(x, W1, b1, W2, b2, Wout, bout)
    res = run_bass_kernel_spmd(
        nc,
        in_maps,
        core_ids=list(range(N_CORES)),
        trace=_trace,
        **(_trace_kwargs or {}),
    )
    out = np.concatenate(
        [res.results[c]["out"] for c in range(N_CORES)]
    ).reshape(B, 1).astype(np.float32)
    if _trace:
        kernel.last_results = res
    return out


if __name__ == "__main__":
    # quick self-run with random data (not the reference distribution)
    rng = np.random.default_rng(0)
    x = (rng.random((B, IN_DIM)) < 32.0 / IN_DIM).astype(np.float32)
    W1 = rng.standard_normal((H1, IN_DIM), dtype=np.float32) / np.sqrt(IN_DIM)
    b1 = rng.standard_normal(H1, dtype=np.float32) / np.sqrt(IN_DIM)
    W2 = rng.standard_normal((H2, H1), dtype=np.float32) / np.sqrt(H1)
    b2 = rng.standard_normal(H2, dtype=np.float32) / np.sqrt(H1)
    Wout = rng.standard_normal((1, H2), dtype=np.float32) / np.sqrt(H2)
    bout = rng.standard_normal(1, dtype=np.float32) / np.sqrt(H2)
    got = kernel(x, W1, b1, W2, b2, Wout, bout)
    h1 = np.maximum(x @ W1.T + b1, 0)
    h2 = np.maximum(h1 @ W2.T + b2, 0)
    exp = h2 @ Wout.T + bout
    print("rel err:", np.abs(got - exp).max() / np.abs(exp).max())


# revision 17
# speedup vs baseline: 6.2393x; 1.0540x over previous
"""HalfKA NNUE forward pass on 8 Trainium2 NeuronCores — sparse gather version.

Network (fp32 reference):
    h1  = relu(x @ W1.T + b1)     x:[2048, 98304] sparse 0/1, W1:[256, 98304]
    h2  = relu(h1 @ W2.T + b2)    W2:[32, 256]
    out = h2 @ Wout.T + bout      Wout:[1, 32]  -> [2048, 1]

x is a few-hot mask (~32 active features per row), so fc1 is an embedding
lookup: h1[b] = sum_{i in active(b)} W1[:, i] + b1. Instead of streaming the
805 MB dense x, the host converts each row to its active-index list and the
device gathers the corresponding 256-dim bf16 embedding columns straight from
HBM with dma_gather (~0.5 MB/core of random 512 B reads).

Sharding: data-parallel over batch — core c owns rows [256c, 256(c+1)), no
collectives. The bf16 embedding table (W1.T) is replicated in every core's
DRAM, split into 4 chunks of 24576 rows (+1 zero pad row each) because
dma_gather indices are int16.

Per core, slots are grouped per (chunk, band-of-128-rows) and padded to a
fixed 1280 (actual max 1116) with zero-row pads. The gathered block
G[slot, emb] for each 128-slot group is reduced into per-row h1 on the PE:
      h1T[emb, row] += G[:, emb].T @ S[:, row]
where S[slot, row] = (rowid[slot] == row) is a one-hot selection matrix built
on the DVE from host-shipped row ids (pads get rowid -1 => zero column). The
result lands directly in the [emb-partition, batch-free] layout that fc2
wants, so bias+relu is a single activation per psum tile and fc2/fc3 are the
same tiny matmuls as the dense kernel.
"""

import sys

sys.path.insert(0, "/opt/trn_rl_repo")

from contextlib import ExitStack

import numpy as np
import ml_dtypes

import concourse.bass as bass  # noqa: F401  (registers engine libraries)
import concourse.tile as tile
from concourse import bacc, mybir
from concourse.bass_utils import run_bass_kernel_spmd

f32 = mybir.dt.float32
bf16 = mybir.dt.bfloat16
i16 = mybir.dt.int16

N_CORES = 8
B = 2048
IN_DIM = 98304
H1 = 256
H2 = 32

RPC = B // N_CORES      # 256 rows per core
BANDS = 2               # 128-row PE bands per core
NCH = 4                 # embedding-table chunks (int16 index range)
CHR = IN_DIM // NCH     # 24576 feature rows per chunk
ZROW = CHR              # zero row appended at the end of each chunk
NPB = 1280              # padded slots per (chunk, band); actual max 1116
GPB = NPB // 128        # 10 groups of 128 slots per band
NPC = NPB * BANDS       # 2560 slots per chunk-gather
GPC = GPB * BANDS       # 20 groups per chunk
MH = H1 // 128          # 2 psum halves of the 256-dim h1

_CACHED = {}


def _build_program():
    nc = bacc.Bacc(
        "TRN2",
        target_bir_lowering=False,
        debug=False,
        num_devices=N_CORES,
        num_swdge_queues=4,
    )

    table = nc.dram_tensor("table", [NCH, CHR + 1, H1], bf16, kind="ExternalInput")
    idx_d = nc.dram_tensor("idx", [128, NCH, NPC // 16], i16, kind="ExternalInput")
    rid_d = nc.dram_tensor("rid", [128, NCH, GPC], bf16, kind="ExternalInput")
    iota_d = nc.dram_tensor("iota", [128, 128], bf16, kind="ExternalInput")
    b1_d = nc.dram_tensor("b1", [128, MH], f32, kind="ExternalInput")
    w2t_d = nc.dram_tensor("w2t", [128, MH, H2], f32, kind="ExternalInput")
    b2_d = nc.dram_tensor("b2", [H2, 1], f32, kind="ExternalInput")
    wout_d = nc.dram_tensor("woutt", [H2 + 1, 1], f32, kind="ExternalInput")
    out = nc.dram_tensor("out", [RPC], f32, kind="ExternalOutput")

    with tile.TileContext(nc) as tc:
        with ExitStack() as ctx:
            const = ctx.enter_context(tc.tile_pool(name="const", bufs=1))
            gp = ctx.enter_context(tc.tile_pool(name="g", bufs=1))
            sp = ctx.enter_context(tc.tile_pool(name="s", bufs=1))
            smp = ctx.enter_context(tc.tile_pool(name="small", bufs=1))
            psa = ctx.enter_context(
                tc.tile_pool(name="psa", bufs=1, space="PSUM")
            )
            ps2 = ctx.enter_context(tc.tile_pool(name="ps2", bufs=1, space="PSUM"))
            ps3 = ctx.enter_context(tc.tile_pool(name="ps3", bufs=1, space="PSUM"))

            idxt = const.tile([128, NCH, NPC // 16], i16)
            nc.sync.dma_start(idxt[:], idx_d.ap())
            ridt = const.tile([128, NCH, GPC], bf16)
            nc.sync.dma_start(ridt[:], rid_d.ap())
            iota = const.tile([128, 128], bf16)
            nc.sync.dma_start(iota[:], iota_d.ap())
            b1_s = const.tile([128, MH], f32)
            nc.sync.dma_start(b1_s[:], b1_d.ap())
            w2t_s = const.tile([128, MH, H2], f32)
            nc.sync.dma_start(w2t_s[:], w2t_d.ap())
            b2_s = const.tile([H2, 1], f32)
            nc.sync.dma_start(b2_s[:], b2_d.ap())
            wout_s = const.tile([H2 + 1, 1], f32)
            nc.sync.dma_start(wout_s[:], wout_d.ap())

            # 3 gathers per table chunk (1024+1024+512 idxs) into one tile:
            # >1024 idxs in one dma_gather overflows the per-queue SWDGE
            # descriptor-ring carveout and deadlocks the ucode's await_space
            # on hardware; 4 queues overlap the latency-bound transfers.
            # slot i -> gt[i % 128, i // 128, :]
            SPLITS = (8, 8, 4)     # 128-slot groups per gather instruction
            gts = []
            nq = 0
            for c in range(NCH):
                gt = gp.tile([128, GPC, H1], bf16, name=f"g{c}", tag=f"g{c}")
                g0 = 0
                for sg in SPLITS:
                    nc.gpsimd.dma_gather(
                        gt[:, g0:g0 + sg, :],
                        table.ap()[c],
                        idxt[:, c, g0 * 8:(g0 + sg) * 8],
                        sg * 128,
                        sg * 128,
                        H1,
                        queue_num=nq % 4,
                    )
                    nq += 1
                    g0 += sg
                gts.append(gt)

            # selection matrices S[c,b][slot, g, row] = (rowid == row)
            sts = {}
            for c in range(NCH):
                for b in range(BANDS):
                    st = sp.tile([128, GPB, 128], bf16, name=f"s{c}_{b}", tag=f"s{c}_{b}")
                    nc.vector.scalar_tensor_tensor(
                        st[:],
                        ridt[:, c, b * GPB:(b + 1) * GPB]
                        .unsqueeze(2)
                        .broadcast_to([128, GPB, 128]),
                        0.0,
                        iota[:].unsqueeze(1).broadcast_to([128, GPB, 128]),
                        mybir.AluOpType.add,
                        mybir.AluOpType.is_equal,
                    )
                    sts[c, b] = st

            # fc1: psum[b][h][emb, row] += G[slot, emb].T @ S[slot, row]
            psum = [
                [psa.tile([128, 128], f32, name=f"ps{b}_{h}") for h in range(MH)]
                for b in range(BANDS)
            ]
            for c in range(NCH):
                for b in range(BANDS):
                    for g in range(GPB):
                        for h in range(MH):
                            nc.tensor.matmul(
                                psum[b][h][:],
                                gts[c][:, b * GPB + g, h * 128:(h + 1) * 128],
                                sts[c, b][:, g, :],
                                start=(c == 0 and g == 0),
                                stop=(c == NCH - 1 and g == GPB - 1),
                            )

            # h1 = relu(fc1 + b1), already transposed: [emb-part, MH, row]
            h1t = smp.tile([128, MH, RPC], f32, name="h1t")
            for b in range(BANDS):
                for h in range(MH):
                    nc.scalar.activation(
                        h1t[:, h, b * 128:(b + 1) * 128],
                        psum[b][h][:],
                        mybir.ActivationFunctionType.Relu,
                        bias=b1_s[:, h:h + 1],
                    )

            # fc2
            p2 = ps2.tile([H2, RPC], f32, name="p2")
            for h in range(MH):
                nc.tensor.matmul(
                    p2[:], w2t_s[:, h, :], h1t[:, h, :],
                    start=(h == 0), stop=(h == MH - 1),
                )
            h2t = smp.tile([H2 + 1, RPC], f32, name="h2t")
            nc.scalar.activation(
                h2t[0:H2, :], p2[:],
                mybir.ActivationFunctionType.Relu,
                bias=b2_s[:],
            )
            nc.vector.memset(h2t[H2:H2 + 1, :], 1.0)

            # fc3 (bout folded in via the ones row)
            p3 = ps3.tile([1, RPC], f32, name="p3")
            nc.tensor.matmul(p3[:], wout_s[:], h2t[:], start=True, stop=True)
            ot = smp.tile([1, RPC], f32, name="ot")
            nc.vector.tensor_copy(ot[:], p3[:])
            nc.sync.dma_start(out.ap(), ot[:])

    nc.compile()
    return nc


def get_program():
    if "nc" not in _CACHED:
        _CACHED["nc"] = _build_program()
    return _CACHED["nc"]


def _prep_inputs(x, W1, b1, W2, b2, Wout, bout):
    """Convert the dense few-hot x into per-core gather index lists and build
    the shared bf16 embedding table + small fc weights."""
    bf = ml_dtypes.bfloat16

    w1T = np.ascontiguousarray(W1.T).astype(bf)             # [IN_DIM, H1]
    table = np.zeros((NCH, CHR + 1, H1), dtype=bf)
    table[:, :CHR, :] = w1T.reshape(NCH, CHR, H1)

    b1_h = np.ascontiguousarray(b1.reshape(MH, 128).T)      # [128, MH]
    w2t_h = np.ascontiguousarray(
        W2.T.reshape(MH, 128, H2).transpose(1, 0, 2)        # [128, MH, H2]
    )
    b2_h = np.ascontiguousarray(b2.reshape(H2, 1)).astype(np.float32)
    wout_h = np.concatenate(
        [Wout.T, bout.reshape(1, 1)], axis=0
    ).astype(np.float32)                                    # [H2+1, 1]
    iota_h = np.ascontiguousarray(
        np.broadcast_to(np.arange(128, dtype=np.float32), (128, 128))
    ).astype(bf)

    rows, cols = np.nonzero(x)                              # row-major sorted
    in_maps = []
    for cidx in range(N_CORES):
        m = (rows >= cidx * RPC) & (rows < (cidx + 1) * RPC)
        r = rows[m] - cidx * RPC
        f = cols[m]
        ch = f // CHR
        band = r // 128

        idx_arr = np.full((NCH, NPC), ZROW, dtype=np.int16)
        rid_arr = np.full((NCH, GPC, 128), -1.0, dtype=np.float32)
        for c in range(NCH):
            for b in range(BANDS):
                sel = (ch == c) & (band == b)
                n = int(sel.sum())
                assert n <= NPB, f"slot padding overflow: {n} > {NPB}"
                # ascending feature order -> DMA descriptors walk increasing
                # HBM addresses (S reassigns slots to rows, any order works)
                order = np.argsort(f[sel], kind="stable")
                pos = b * NPB + np.arange(n)
                idx_arr[c, pos] = (f[sel][order] - c * CHR).astype(np.int16)
                rid_arr[c, pos // 128, pos % 128] = r[sel][order] - b * 128

        # dma_gather reads slot i's index at idxs[i % 16, i // 16], replicated
        # across the eight 16-partition gpsimd cores
        w = idx_arr.reshape(NCH, NPC // 16, 16)             # [c, s, j]
        idx_t = np.ascontiguousarray(
            np.tile(w.transpose(2, 0, 1), (8, 1, 1))        # [128, c, s]
        )
        rid_t = np.ascontiguousarray(
            rid_arr.transpose(2, 0, 1).astype(bf)           # [128, NCH, GPC]
        )
        in_maps.append({
            "table": table,
            "idx": idx_t,
            "rid": rid_t,
            "iota": iota_h,
            "b1": b1_h,
            "w2t": w2t_h,
            "b2": b2_h,
            "woutt": wout_h,
        })
    return in_maps


def kernel(x, W1, b1, W2, b2, Wout, bout, _trace=False, _trace_kwargs=None):
    x = np.asarray(x, dtype=np.float32)
    W1 = np.asarray(W1, dtype=np.float32)
    b1 = np.asarray(b1, dtype=np.float32)
    W2 = np.asarray(W2, dtype=np.float32)
    b2 = np.asarray(b2, dtype=np.float32)
    Wout = np.asarray(Wout, dtype=np.float32)
    bout = np.asarray(bout, dtype=np.float32)

    nc = get_program()
    in_maps = _prep_inputs(x, W1, b1, W2, b2, Wout, bout)
    res = run_bass_kernel_spmd(
        nc,
        in_maps,
        core_ids=list(range(N_CORES)),
        trace=_trace,
        **(_trace_kwargs or {}),
    )
    out = np.concatenate(
        [res.results[c]["out"] for c in range(N_CORES)]
    ).reshape(B, 1).astype(np.float32)
    if _trace:
        kernel.last_results = res
    return out


if __name__ == "__main__":
    # quick self-run with random data (not the reference distribution)
    rng = np.random.default_rng(0)
    x = (rng.random((B, IN_DIM)) < 32.0 / IN_DIM).astype(np.float32)
    W1 = rng.standard_normal((H1, IN_DIM), dtype=np.float32) / np.sqrt(IN_DIM)
    b1 = rng.standard_normal(H1, dtype=np.float32) / np.sqrt(IN_DIM)
    W2 = rng.standard_normal((H2, H1), dtype=np.float32) / np.sqrt(H1)
    b2 = rng.standard_normal(H2, dtype=np.float32) / np.sqrt(H1)
    Wout = rng.standard_normal((1, H2), dtype=np.float32) / np.sqrt(H2)
    bout = rng.standard_normal(1, dtype=np.float32) / np.sqrt(H2)
    got = kernel(x, W1, b1, W2, b2, Wout, bout)
    h1 = np.maximum(x @ W1.T + b1, 0)
    h2 = np.maximum(h1 @ W2.T + b2, 0)
    exp = h2 @ Wout.T + bout
    print("rel err:", np.abs(got - exp).max() / np.abs(exp).max())


# revision 22
# speedup vs baseline: 6.3247x; 1.0137x over previous
"""HalfKA NNUE forward pass on 8 Trainium2 NeuronCores — sparse gather version.

Network (fp32 reference):
    h1  = relu(x @ W1.T + b1)     x:[2048, 98304] sparse 0/1, W1:[256, 98304]
    h2  = relu(h1 @ W2.T + b2)    W2:[32, 256]
    out = h2 @ Wout.T + bout      Wout:[1, 32]  -> [2048, 1]

x is a few-hot mask (~32 active features per row), so fc1 is an embedding
lookup: h1[b] = sum_{i in active(b)} W1[:, i] + b1. Instead of streaming the
805 MB dense x, the host converts each row to its active-index list and the
device gathers the corresponding 256-dim bf16 embedding columns straight from
HBM with dma_gather (~0.5 MB/core of random 512 B reads).

Sharding: data-parallel over batch — core c owns rows [256c, 256(c+1)), no
collectives. The bf16 embedding table (W1.T) is replicated in every core's
DRAM, split into 4 chunks of 24576 rows (+1 zero pad row each) because
dma_gather indices are int16.

Per core, slots are grouped per (chunk, band-of-128-rows) and padded to a
fixed 1280 (actual max 1116) with zero-row pads. The gathered block
G[slot, emb] for each 128-slot group is reduced into per-row h1 on the PE:
      h1T[emb, row] += G[:, emb].T @ S[:, row]
where S[slot, row] = (rowid[slot] == row) is a one-hot selection matrix built
on the DVE from host-shipped row ids (pads get rowid -1 => zero column). The
result lands directly in the [emb-partition, batch-free] layout that fc2
wants, so bias+relu is a single activation per psum tile and fc2/fc3 are the
same tiny matmuls as the dense kernel.
"""

import sys

sys.path.insert(0, "/opt/trn_rl_repo")

from contextlib import ExitStack

import numpy as np
import ml_dtypes

import concourse.bass as bass  # noqa: F401  (registers engine libraries)
import concourse.tile as tile
from concourse import bacc, mybir
from concourse.bass_utils import run_bass_kernel_spmd

f32 = mybir.dt.float32
bf16 = mybir.dt.bfloat16
i16 = mybir.dt.int16

N_CORES = 8
B = 2048
IN_DIM = 98304
H1 = 256
H2 = 32

RPC = B // N_CORES      # 256 rows per core
BANDS = 2               # 128-row PE bands per core
NCH = 4                 # embedding-table chunks (int16 index range)
CHR = IN_DIM // NCH     # 24576 feature rows per chunk
ZROW = CHR              # zero row appended at the end of each chunk
NPB = 1280              # padded slots per (chunk, band); actual max 1116
GPB = NPB // 128        # 10 groups of 128 slots per band
NPC = NPB * BANDS       # 2560 slots per chunk-gather
GPC = GPB * BANDS       # 20 groups per chunk
MH = H1 // 128          # 2 psum halves of the 256-dim h1

_CACHED = {}


def _build_program():
    nc = bacc.Bacc(
        "TRN2",
        target_bir_lowering=False,
        debug=False,
        num_devices=N_CORES,
        num_swdge_queues=4,
    )

    table = nc.dram_tensor("table", [NCH, CHR + 1, H1], bf16, kind="ExternalInput")
    idx_d = nc.dram_tensor("idx", [128, NCH, NPC // 16], i16, kind="ExternalInput")
    rid_d = nc.dram_tensor("rid", [128, NCH, GPC], bf16, kind="ExternalInput")
    iota_d = nc.dram_tensor("iota", [128, 128], bf16, kind="ExternalInput")
    b1_d = nc.dram_tensor("b1", [128, MH], f32, kind="ExternalInput")
    w2t_d = nc.dram_tensor("w2t", [128, MH, H2], f32, kind="ExternalInput")
    b2_d = nc.dram_tensor("b2", [H2, 1], f32, kind="ExternalInput")
    wout_d = nc.dram_tensor("woutt", [H2 + 1, 1], f32, kind="ExternalInput")
    out = nc.dram_tensor("out", [RPC], f32, kind="ExternalOutput")

    with tile.TileContext(nc) as tc:
        with ExitStack() as ctx:
            const = ctx.enter_context(tc.tile_pool(name="const", bufs=1))
            gp = ctx.enter_context(tc.tile_pool(name="g", bufs=1))
            sp = ctx.enter_context(tc.tile_pool(name="s", bufs=1))
            smp = ctx.enter_context(tc.tile_pool(name="small", bufs=1))
            psa = ctx.enter_context(
                tc.tile_pool(name="psa", bufs=1, space="PSUM")
            )
            ps2 = ctx.enter_context(tc.tile_pool(name="ps2", bufs=1, space="PSUM"))
            ps3 = ctx.enter_context(tc.tile_pool(name="ps3", bufs=1, space="PSUM"))

            idxt = const.tile([128, NCH, NPC // 16], i16)
            nc.sync.dma_start(idxt[:], idx_d.ap())
            ridt = const.tile([128, NCH, GPC], bf16)
            nc.sync.dma_start(ridt[:], rid_d.ap())
            iota = const.tile([128, 128], bf16)
            nc.sync.dma_start(iota[:], iota_d.ap())
            b1_s = const.tile([128, MH], f32)
            nc.sync.dma_start(b1_s[:], b1_d.ap())
            w2t_s = const.tile([128, MH, H2], f32)
            nc.scalar.dma_start(w2t_s[:], w2t_d.ap())
            b2_s = const.tile([H2, 1], f32)
            nc.scalar.dma_start(b2_s[:], b2_d.ap())
            wout_s = const.tile([H2 + 1, 1], f32)
            nc.scalar.dma_start(wout_s[:], wout_d.ap())

            # 3 gathers per table chunk (1024+1024+512 idxs) into one tile:
            # >1024 idxs in one dma_gather overflows the per-queue SWDGE
            # descriptor-ring carveout and deadlocks the ucode's await_space
            # on hardware; 4 queues overlap the latency-bound transfers.
            # slot i -> gt[i % 128, i // 128, :]
            SPLITS = (8, 8, 4)     # 128-slot groups per gather instruction
            SPLIT_OFF = (0, 8, 16)
            gts = {}               # (chunk, split) -> tile, for fine deps:
            nq = 0                 # matmuls start when their split lands
            for c in range(NCH):
                g0 = 0
                for v, sg in enumerate(SPLITS):
                    gt = gp.tile([128, sg, H1], bf16, name=f"g{c}_{v}",
                                 tag=f"g{c}_{v}")
                    nc.gpsimd.dma_gather(
                        gt[:],
                        table.ap()[c],
                        idxt[:, c, g0 * 8:(g0 + sg) * 8],
                        sg * 128,
                        sg * 128,
                        H1,
                        queue_num=nq % 4,
                    )
                    nq += 1
                    g0 += sg
                    gts[c, v] = gt

            # selection matrices S[c,b][slot, g, row] = (rowid == row)
            sts = {}
            for c in range(NCH):
                for b in range(BANDS):
                    st = sp.tile([128, GPB, 128], bf16, name=f"s{c}_{b}", tag=f"s{c}_{b}")
                    nc.vector.scalar_tensor_tensor(
                        st[:],
                        ridt[:, c, b * GPB:(b + 1) * GPB]
                        .unsqueeze(2)
                        .broadcast_to([128, GPB, 128]),
                        0.0,
                        iota[:].unsqueeze(1).broadcast_to([128, GPB, 128]),
                        mybir.AluOpType.add,
                        mybir.AluOpType.is_equal,
                    )
                    sts[c, b] = st

            # fc1: psum[b][h][emb, row] += G[slot, emb].T @ S[slot, row]
            psum = [
                [psa.tile([128, 128], f32, name=f"ps{b}_{h}") for h in range(MH)]
                for b in range(BANDS)
            ]
            for c in range(NCH):
                for b in range(BANDS):
                    for g in range(GPB):
                        gg = b * GPB + g          # slot group within chunk
                        v = 0 if gg < 8 else (1 if gg < 16 else 2)
                        for h in range(MH):
                            nc.tensor.matmul(
                                psum[b][h][:],
                                gts[c, v][:, gg - SPLIT_OFF[v],
                                          h * 128:(h + 1) * 128],
                                sts[c, b][:, g, :],
                                start=(c == 0 and g == 0),
                                stop=(c == NCH - 1 and g == GPB - 1),
                            )

            # per-band tail: relu+bias (h1 already [emb-part, row]), fc2,
            # relu+b2 — band 0's tail overlaps band 1's last fc1 matmuls
            h1t = smp.tile([128, MH, RPC], f32, name="h1t")
            p2 = ps2.tile([H2, RPC], f32, name="p2")
            h2t = smp.tile([H2 + 1, RPC], f32, name="h2t")
            for b in range(BANDS):
                for h in range(MH):
                    nc.scalar.activation(
                        h1t[:, h, b * 128:(b + 1) * 128],
                        psum[b][h][:],
                        mybir.ActivationFunctionType.Relu,
                        bias=b1_s[:, h:h + 1],
                    )
                for h in range(MH):
                    nc.tensor.matmul(
                        p2[:, b * 128:(b + 1) * 128],
                        w2t_s[:, h, :],
                        h1t[:, h, b * 128:(b + 1) * 128],
                        start=(h == 0), stop=(h == MH - 1),
                    )
                nc.scalar.activation(
                    h2t[0:H2, b * 128:(b + 1) * 128],
                    p2[:, b * 128:(b + 1) * 128],
                    mybir.ActivationFunctionType.Relu,
                    bias=b2_s[:],
                )
            nc.vector.memset(h2t[H2:H2 + 1, :], 1.0)

            # fc3 (bout folded in via the ones row)
            p3 = ps3.tile([1, RPC], f32, name="p3")
            nc.tensor.matmul(p3[:], wout_s[:], h2t[:], start=True, stop=True)
            ot = smp.tile([1, RPC], f32, name="ot")
            nc.vector.tensor_copy(ot[:], p3[:])
            nc.sync.dma_start(out.ap(), ot[:])

    nc.compile()
    return nc


def get_program():
    if "nc" not in _CACHED:
        _CACHED["nc"] = _build_program()
    return _CACHED["nc"]


def _prep_inputs(x, W1, b1, W2, b2, Wout, bout):
    """Convert the dense few-hot x into per-core gather index lists and build
    the shared bf16 embedding table + small fc weights."""
    bf = ml_dtypes.bfloat16

    w1T = np.ascontiguousarray(W1.T).astype(bf)             # [IN_DIM, H1]
    table = np.zeros((NCH, CHR + 1, H1), dtype=bf)
    table[:, :CHR, :] = w1T.reshape(NCH, CHR, H1)

    b1_h = np.ascontiguousarray(b1.reshape(MH, 128).T)      # [128, MH]
    w2t_h = np.ascontiguousarray(
        W2.T.reshape(MH, 128, H2).transpose(1, 0, 2)        # [128, MH, H2]
    )
    b2_h = np.ascontiguousarray(b2.reshape(H2, 1)).astype(np.float32)
    wout_h = np.concatenate(
        [Wout.T, bout.reshape(1, 1)], axis=0
    ).astype(np.float32)                                    # [H2+1, 1]
    iota_h = np.ascontiguousarray(
        np.broadcast_to(np.arange(128, dtype=np.float32), (128, 128))
    ).astype(bf)

    rows, cols = np.nonzero(x)                              # row-major sorted
    in_maps = []
    for cidx in range(N_CORES):
        m = (rows >= cidx * RPC) & (rows < (cidx + 1) * RPC)
        r = rows[m] - cidx * RPC
        f = cols[m]
        ch = f // CHR
        band = r // 128

        idx_arr = np.full((NCH, NPC), ZROW, dtype=np.int16)
        rid_arr = np.full((NCH, GPC, 128), -1.0, dtype=np.float32)
        for c in range(NCH):
            for b in range(BANDS):
                sel = (ch == c) & (band == b)
                n = int(sel.sum())
                assert n <= NPB, f"slot padding overflow: {n} > {NPB}"
                # ascending feature order -> DMA descriptors walk increasing
                # HBM addresses (S reassigns slots to rows, any order works)
                order = np.argsort(f[sel], kind="stable")
                pos = b * NPB + np.arange(n)
                idx_arr[c, pos] = (f[sel][order] - c * CHR).astype(np.int16)
                rid_arr[c, pos // 128, pos % 128] = r[sel][order] - b * 128

        # dma_gather reads slot i's index at idxs[i % 16, i // 16], replicated
        # across the eight 16-partition gpsimd cores
        w = idx_arr.reshape(NCH, NPC // 16, 16)             # [c, s, j]
        idx_t = np.ascontiguousarray(
            np.tile(w.transpose(2, 0, 1), (8, 1, 1))        # [128, c, s]
        )
        rid_t = np.ascontiguousarray(
            rid_arr.transpose(2, 0, 1).astype(bf)           # [128, NCH, GPC]
        )
        in_maps.append({
            "table": table,
            "idx": idx_t,
            "rid": rid_t,
            "iota": iota_h,
            "b1": b1_h,
            "w2t": w2t_h,
            "b2": b2_h,
            "woutt": wout_h,
        })
    return in_maps


def kernel(x, W1, b1, W2, b2, Wout, bout, _trace=False, _trace_kwargs=None):
    x = np.asarray(x, dtype=np.float32)
    W1 = np.asarray(W1, dtype=np.float32)
    b1 = np.asarray(b1, dtype=np.float32)
    W2 = np.asarray(W2, dtype=np.float32)
    b2 = np.asarray(b2, dtype=np.float32)
    Wout = np.asarray(Wout, dtype=np.float32)
    bout = np.asarray(bout, dtype=np.float32)

    nc = get_program()
    in_maps = _prep_inputs(x, W1, b1, W2, b2, Wout, bout)
    res = run_bass_kernel_spmd(
        nc,
        in_maps,
        core_ids=list(range(N_CORES)),
        trace=_trace,
        **(_trace_kwargs or {}),
    )
    out = np.concatenate(
        [res.results[c]["out"] for c in range(N_CORES)]
    ).reshape(B, 1).astype(np.float32)
    if _trace:
        kernel.last_results = res
    return out


if __name__ == "__main__":
    # quick self-run with random data (not the reference distribution)
    rng = np.random.default_rng(0)
    x = (rng.random((B, IN_DIM)) < 32.0 / IN_DIM).astype(np.float32)
    W1 = rng.standard_normal((H1, IN_DIM), dtype=np.float32) / np.sqrt(IN_DIM)
    b1 = rng.standard_normal(H1, dtype=np.float32) / np.sqrt(IN_DIM)
    W2 = rng.standard_normal((H2, H1), dtype=np.float32) / np.sqrt(H1)
    b2 = rng.standard_normal(H2, dtype=np.float32) / np.sqrt(H1)
    Wout = rng.standard_normal((1, H2), dtype=np.float32) / np.sqrt(H2)
    bout = rng.standard_normal(1, dtype=np.float32) / np.sqrt(H2)
    got = kernel(x, W1, b1, W2, b2, Wout, bout)
    h1 = np.maximum(x @ W1.T + b1, 0)
    h2 = np.maximum(h1 @ W2.T + b2, 0)
    exp = h2 @ Wout.T + bout
    print("rel err:", np.abs(got - exp).max() / np.abs(exp).max())


# revision 24
# speedup vs baseline: 6.5916x; 1.0422x over previous
"""HalfKA NNUE forward pass on 8 Trainium2 NeuronCores — sparse gather version.

Network (fp32 reference):
    h1  = relu(x @ W1.T + b1)     x:[2048, 98304] sparse 0/1, W1:[256, 98304]
    h2  = relu(h1 @ W2.T + b2)    W2:[32, 256]
    out = h2 @ Wout.T + bout      Wout:[1, 32]  -> [2048, 1]

x is a few-hot mask (~32 active features per row), so fc1 is an embedding
lookup: h1[b] = sum_{i in active(b)} W1[:, i] + b1. Instead of streaming the
805 MB dense x, the host converts each row to its active-index list and the
device gathers the corresponding 256-dim bf16 embedding columns straight from
HBM with dma_gather (~0.5 MB/core of random 512 B reads).

Sharding: data-parallel over batch — core c owns rows [256c, 256(c+1)), no
collectives. The bf16 embedding table (W1.T) is replicated in every core's
DRAM, split into 4 chunks of 24576 rows (+1 zero pad row each) because
dma_gather indices are int16.

Per core, slots are grouped per (chunk, band-of-128-rows) and padded to a
fixed 1280 (actual max 1116) with zero-row pads. The gathered block
G[slot, emb] for each 128-slot group is reduced into per-row h1 on the PE:
      h1T[emb, row] += G[:, emb].T @ S[:, row]
where S[slot, row] = (rowid[slot] == row) is a one-hot selection matrix built
on the DVE from host-shipped row ids (pads get rowid -1 => zero column). The
result lands directly in the [emb-partition, batch-free] layout that fc2
wants, so bias+relu is a single activation per psum tile and fc2/fc3 are the
same tiny matmuls as the dense kernel.
"""

import sys

sys.path.insert(0, "/opt/trn_rl_repo")

from contextlib import ExitStack

import numpy as np
import ml_dtypes

import concourse.bass as bass  # noqa: F401  (registers engine libraries)
import concourse.tile as tile
from concourse import bacc, mybir
from concourse.bass_utils import run_bass_kernel_spmd

f32 = mybir.dt.float32
bf16 = mybir.dt.bfloat16
i16 = mybir.dt.int16

N_CORES = 8
B = 2048
IN_DIM = 98304
H1 = 256
H2 = 32

RPC = B // N_CORES      # 256 rows per core
BANDS = 2               # 128-row PE bands per core
NCH = 4                 # embedding-table chunks (int16 index range)
CHR = IN_DIM // NCH     # 24576 feature rows per chunk
ZROW = CHR              # zero row appended at the end of each chunk
NPB = 1280              # padded slots per (chunk, band); actual max 1116
GPB = NPB // 128        # 10 groups of 128 slots per band
NPC = NPB * BANDS       # 2560 slots per chunk-gather
GPC = GPB * BANDS       # 20 groups per chunk
MH = H1 // 128          # 2 psum halves of the 256-dim h1

_CACHED = {}


def _build_program():
    nc = bacc.Bacc(
        "TRN2",
        target_bir_lowering=False,
        debug=False,
        num_devices=N_CORES,
        num_swdge_queues=4,
    )

    table = nc.dram_tensor("table", [NCH, CHR + 1, H1], bf16, kind="ExternalInput")
    idx_d = nc.dram_tensor("idx", [128, NCH, NPC // 16], i16, kind="ExternalInput")
    rid_d = nc.dram_tensor("rid", [128, NCH, GPC], bf16, kind="ExternalInput")
    iota_d = nc.dram_tensor("iota", [128, 128], bf16, kind="ExternalInput")
    b1_d = nc.dram_tensor("b1", [128, MH], f32, kind="ExternalInput")
    w2t_d = nc.dram_tensor("w2t", [128, MH, H2], f32, kind="ExternalInput")
    b2_d = nc.dram_tensor("b2", [H2, 1], f32, kind="ExternalInput")
    wout_d = nc.dram_tensor("woutt", [H2 + 1, 1], f32, kind="ExternalInput")
    out = nc.dram_tensor("out", [RPC], f32, kind="ExternalOutput")

    with tile.TileContext(nc) as tc:
        with ExitStack() as ctx:
            const = ctx.enter_context(tc.tile_pool(name="const", bufs=1))
            gp = ctx.enter_context(tc.tile_pool(name="g", bufs=1))
            sp = ctx.enter_context(tc.tile_pool(name="s", bufs=1))
            smp = ctx.enter_context(tc.tile_pool(name="small", bufs=1))
            psa = ctx.enter_context(
                tc.tile_pool(name="psa", bufs=1, space="PSUM")
            )
            ps2 = ctx.enter_context(tc.tile_pool(name="ps2", bufs=1, space="PSUM"))
            ps3 = ctx.enter_context(tc.tile_pool(name="ps3", bufs=1, space="PSUM"))

            idxt = const.tile([128, NCH, NPC // 16], i16)
            nc.sync.dma_start(idxt[:], idx_d.ap())
            ridt = const.tile([128, NCH, GPC], bf16)
            nc.sync.dma_start(ridt[:], rid_d.ap())
            iota = const.tile([128, 128], bf16)
            nc.sync.dma_start(iota[:], iota_d.ap())
            b1_s = const.tile([128, MH], f32)
            nc.sync.dma_start(b1_s[:], b1_d.ap())
            w2t_s = const.tile([128, MH, H2], f32)
            nc.scalar.dma_start(w2t_s[:], w2t_d.ap())
            b2_s = const.tile([H2, 1], f32)
            nc.scalar.dma_start(b2_s[:], b2_d.ap())
            wout_s = const.tile([H2 + 1, 1], f32)
            nc.scalar.dma_start(wout_s[:], wout_d.ap())

            # 3 gathers per table chunk (1024+1024+512 idxs) into one tile:
            # >1024 idxs in one dma_gather overflows the per-queue SWDGE
            # descriptor-ring carveout and deadlocks the ucode's await_space
            # on hardware; 4 queues overlap the latency-bound transfers.
            # slot i -> gt[i % 128, i // 128, :]
            # chunk 0 leads with a small gather so its DMA sem fires early
            # and the PE (end-critical) starts ~9us sooner
            CSPLITS = {0: (4, 8, 8), 1: (8, 8, 4), 2: (8, 8, 4), 3: (8, 8, 4)}
            COFF = {c: (0, s[0], s[0] + s[1]) for c, s in CSPLITS.items()}
            gts = {}               # (chunk, split) -> tile, for fine deps:
            nq = 0                 # matmuls start when their split lands
            for c in range(NCH):
                g0 = 0
                for v, sg in enumerate(CSPLITS[c]):
                    gt = gp.tile([128, sg, H1], bf16, name=f"g{c}_{v}",
                                 tag=f"g{c}_{v}")
                    nc.gpsimd.dma_gather(
                        gt[:],
                        table.ap()[c],
                        idxt[:, c, g0 * 8:(g0 + sg) * 8],
                        sg * 128,
                        sg * 128,
                        H1,
                        queue_num=nq % 4,
                    )
                    nq += 1
                    g0 += sg
                    gts[c, v] = gt

            # selection matrices S[c,b][slot, g, row] = (rowid == row)
            sts = {}
            for c in range(NCH):
                for b in range(BANDS):
                    st = sp.tile([128, GPB, 128], bf16, name=f"s{c}_{b}", tag=f"s{c}_{b}")
                    nc.vector.scalar_tensor_tensor(
                        st[:],
                        ridt[:, c, b * GPB:(b + 1) * GPB]
                        .unsqueeze(2)
                        .broadcast_to([128, GPB, 128]),
                        0.0,
                        iota[:].unsqueeze(1).broadcast_to([128, GPB, 128]),
                        mybir.AluOpType.add,
                        mybir.AluOpType.is_equal,
                    )
                    sts[c, b] = st

            # fc1: psum[b][h][emb, row] += G[slot, emb].T @ S[slot, row]
            psum = [
                [psa.tile([128, 128], f32, name=f"ps{b}_{h}") for h in range(MH)]
                for b in range(BANDS)
            ]
            for c in range(NCH):
                for b in range(BANDS):
                    for g in range(GPB):
                        gg = b * GPB + g          # slot group within chunk
                        off = COFF[c]
                        v = 0 if gg < off[1] else (1 if gg < off[2] else 2)
                        for h in range(MH):
                            nc.tensor.matmul(
                                psum[b][h][:],
                                gts[c, v][:, gg - off[v],
                                          h * 128:(h + 1) * 128],
                                sts[c, b][:, g, :],
                                start=(c == 0 and g == 0),
                                stop=(c == NCH - 1 and g == GPB - 1),
                            )

            # per-band tail: relu+bias (h1 already [emb-part, row]), fc2,
            # relu+b2 — band 0's tail overlaps band 1's last fc1 matmuls
            h1t = smp.tile([128, MH, RPC], f32, name="h1t")
            p2 = ps2.tile([H2, RPC], f32, name="p2")
            h2t = smp.tile([H2 + 1, RPC], f32, name="h2t")
            for b in range(BANDS):
                for h in range(MH):
                    nc.scalar.activation(
                        h1t[:, h, b * 128:(b + 1) * 128],
                        psum[b][h][:],
                        mybir.ActivationFunctionType.Relu,
                        bias=b1_s[:, h:h + 1],
                    )
                for h in range(MH):
                    nc.tensor.matmul(
                        p2[:, b * 128:(b + 1) * 128],
                        w2t_s[:, h, :],
                        h1t[:, h, b * 128:(b + 1) * 128],
                        start=(h == 0), stop=(h == MH - 1),
                    )
                nc.scalar.activation(
                    h2t[0:H2, b * 128:(b + 1) * 128],
                    p2[:, b * 128:(b + 1) * 128],
                    mybir.ActivationFunctionType.Relu,
                    bias=b2_s[:],
                )
            nc.vector.memset(h2t[H2:H2 + 1, :], 1.0)

            # fc3 (bout folded in via the ones row)
            p3 = ps3.tile([1, RPC], f32, name="p3")
            nc.tensor.matmul(p3[:], wout_s[:], h2t[:], start=True, stop=True)
            ot = smp.tile([1, RPC], f32, name="ot")
            nc.vector.tensor_copy(ot[:], p3[:])
            nc.sync.dma_start(out.ap(), ot[:])

    nc.compile()
    return nc


def get_program():
    if "nc" not in _CACHED:
        _CACHED["nc"] = _build_program()
    return _CACHED["nc"]


def _prep_inputs(x, W1, b1, W2, b2, Wout, bout):
    """Convert the dense few-hot x into per-core gather index lists and build
    the shared bf16 embedding table + small fc weights."""
    bf = ml_dtypes.bfloat16

    w1T = np.ascontiguousarray(W1.T).astype(bf)             # [IN_DIM, H1]
    table = np.zeros((NCH, CHR + 1, H1), dtype=bf)
    table[:, :CHR, :] = w1T.reshape(NCH, CHR, H1)

    b1_h = np.ascontiguousarray(b1.reshape(MH, 128).T)      # [128, MH]
    w2t_h = np.ascontiguousarray(
        W2.T.reshape(MH, 128, H2).transpose(1, 0, 2)        # [128, MH, H2]
    )
    b2_h = np.ascontiguousarray(b2.reshape(H2, 1)).astype(np.float32)
    wout_h = np.concatenate(
        [Wout.T, bout.reshape(1, 1)], axis=0
    ).astype(np.float32)                                    # [H2+1, 1]
    iota_h = np.ascontiguousarray(
        np.broadcast_to(np.arange(128, dtype=np.float32), (128, 128))
    ).astype(bf)

    rows, cols = np.nonzero(x)                              # row-major sorted
    in_maps = []
    for cidx in range(N_CORES):
        m = (rows >= cidx * RPC) & (rows < (cidx + 1) * RPC)
        r = rows[m] - cidx * RPC
        f = cols[m]
        ch = f // CHR
        band = r // 128

        idx_arr = np.full((NCH, NPC), ZROW, dtype=np.int16)
        rid_arr = np.full((NCH, GPC, 128), -1.0, dtype=np.float32)
        for c in range(NCH):
            for b in range(BANDS):
                sel = (ch == c) & (band == b)
                n = int(sel.sum())
                assert n <= NPB, f"slot padding overflow: {n} > {NPB}"
                # ascending feature order -> DMA descriptors walk increasing
                # HBM addresses (S reassigns slots to rows, any order works)
                order = np.argsort(f[sel], kind="stable")
                pos = b * NPB + np.arange(n)
                idx_arr[c, pos] = (f[sel][order] - c * CHR).astype(np.int16)
                rid_arr[c, pos // 128, pos % 128] = r[sel][order] - b * 128

        # dma_gather reads slot i's index at idxs[i % 16, i // 16], replicated
        # across the eight 16-partition gpsimd cores
        w = idx_arr.reshape(NCH, NPC // 16, 16)             # [c, s, j]
        idx_t = np.ascontiguousarray(
            np.tile(w.transpose(2, 0, 1), (8, 1, 1))        # [128, c, s]
        )
        rid_t = np.ascontiguousarray(
            rid_arr.transpose(2, 0, 1).astype(bf)           # [128, NCH, GPC]
        )
        in_maps.append({
            "table": table,
            "idx": idx_t,
            "rid": rid_t,
            "iota": iota_h,
            "b1": b1_h,
            "w2t": w2t_h,
            "b2": b2_h,
            "woutt": wout_h,
        })
    return in_maps


def kernel(x, W1, b1, W2, b2, Wout, bout, _trace=False, _trace_kwargs=None):
    x = np.asarray(x, dtype=np.float32)
    W1 = np.asarray(W1, dtype=np.float32)
    b1 = np.asarray(b1, dtype=np.float32)
    W2 = np.asarray(W2, dtype=np.float32)
    b2 = np.asarray(b2, dtype=np.float32)
    Wout = np.asarray(Wout, dtype=np.float32)
    bout = np.asarray(bout, dtype=np.float32)

    nc = get_program()
    in_maps = _prep_inputs(x, W1, b1, W2, b2, Wout, bout)
    res = run_bass_kernel_spmd(
        nc,
        in_maps,
        core_ids=list(range(N_CORES)),
        trace=_trace,
        **(_trace_kwargs or {}),
    )
    out = np.concatenate(
        [res.results[c]["out"] for c in range(N_CORES)]
    ).reshape(B, 1).astype(np.float32)
    if _trace:
        kernel.last_results = res
    return out


if __name__ == "__main__":
    # quick self-run with random data (not the reference distribution)
    rng = np.random.default_rng(0)
    x = (rng.random((B, IN_DIM)) < 32.0 / IN_DIM).astype(np.float32)
    W1 = rng.standard_normal((H1, IN_DIM), dtype=np.float32) / np.sqrt(IN_DIM)
    b1 = rng.standard_normal(H1, dtype=np.float32) / np.sqrt(IN_DIM)
    W2 = rng.standard_normal((H2, H1), dtype=np.float32) / np.sqrt(H1)
    b2 = rng.standard_normal(H2, dtype=np.float32) / np.sqrt(H1)
    Wout = rng.standard_normal((1, H2), dtype=np.float32) / np.sqrt(H2)
    bout = rng.standard_normal(1, dtype=np.float32) / np.sqrt(H2)
    got = kernel(x, W1, b1, W2, b2, Wout, bout)
    h1 = np.maximum(x @ W1.T + b1, 0)
    h2 = np.maximum(h1 @ W2.T + b2, 0)
    exp = h2 @ Wout.T + bout
    print("rel err:", np.abs(got - exp).max() / np.abs(exp).max())


# revision 25
# speedup vs baseline: 7.0008x; 1.0621x over previous
"""HalfKA NNUE forward pass on 8 Trainium2 NeuronCores — sparse gather version.

Network (fp32 reference):
    h1  = relu(x @ W1.T + b1)     x:[2048, 98304] sparse 0/1, W1:[256, 98304]
    h2  = relu(h1 @ W2.T + b2)    W2:[32, 256]
    out = h2 @ Wout.T + bout      Wout:[1, 32]  -> [2048, 1]

x is a few-hot mask (~32 active features per row), so fc1 is an embedding
lookup: h1[b] = sum_{i in active(b)} W1[:, i] + b1. Instead of streaming the
805 MB dense x, the host converts each row to its active-index list and the
device gathers the corresponding 256-dim bf16 embedding columns straight from
HBM with dma_gather (~0.5 MB/core of random 512 B reads).

Sharding: data-parallel over batch — core c owns rows [256c, 256(c+1)), no
collectives. The bf16 embedding table (W1.T) is replicated in every core's
DRAM, split into 4 chunks of 24576 rows (+1 zero pad row each) because
dma_gather indices are int16.

Per core, slots are grouped per (chunk, band-of-128-rows) and padded to a
fixed 1280 (actual max 1116) with zero-row pads. The gathered block
G[slot, emb] for each 128-slot group is reduced into per-row h1 on the PE:
      h1T[emb, row] += G[:, emb].T @ S[:, row]
where S[slot, row] = (rowid[slot] == row) is a one-hot selection matrix built
on the DVE from host-shipped row ids (pads get rowid -1 => zero column). The
result lands directly in the [emb-partition, batch-free] layout that fc2
wants, so bias+relu is a single activation per psum tile and fc2/fc3 are the
same tiny matmuls as the dense kernel.
"""

import sys

sys.path.insert(0, "/opt/trn_rl_repo")

from contextlib import ExitStack

import numpy as np
import ml_dtypes

import concourse.bass as bass  # noqa: F401  (registers engine libraries)
import concourse.tile as tile
from concourse import bacc, mybir
from concourse.bass_utils import run_bass_kernel_spmd

f32 = mybir.dt.float32
bf16 = mybir.dt.bfloat16
i16 = mybir.dt.int16

N_CORES = 8
B = 2048
IN_DIM = 98304
H1 = 256
H2 = 32

RPC = B // N_CORES      # 256 rows per core
BANDS = 2               # 128-row PE bands per core
NCH = 4                 # embedding-table chunks (int16 index range)
CHR = IN_DIM // NCH     # 24576 feature rows per chunk
ZROW = CHR              # zero row appended at the end of each chunk
NPB = 1152              # padded slots per (chunk, band); actual max 1116
GPB = NPB // 128        # 10 groups of 128 slots per band
NPC = NPB * BANDS       # 2560 slots per chunk-gather
GPC = GPB * BANDS       # 20 groups per chunk
MH = H1 // 128          # 2 psum halves of the 256-dim h1

_CACHED = {}


def _build_program():
    nc = bacc.Bacc(
        "TRN2",
        target_bir_lowering=False,
        debug=False,
        num_devices=N_CORES,
        num_swdge_queues=4,
    )

    table = nc.dram_tensor("table", [NCH, CHR + 1, H1], bf16, kind="ExternalInput")
    idx_d = nc.dram_tensor("idx", [128, NCH, NPC // 16], i16, kind="ExternalInput")
    rid_d = nc.dram_tensor("rid", [128, NCH, GPC], bf16, kind="ExternalInput")
    iota_d = nc.dram_tensor("iota", [128, 128], bf16, kind="ExternalInput")
    b1_d = nc.dram_tensor("b1", [128, MH], f32, kind="ExternalInput")
    w2t_d = nc.dram_tensor("w2t", [128, MH, H2], f32, kind="ExternalInput")
    b2_d = nc.dram_tensor("b2", [H2, 1], f32, kind="ExternalInput")
    wout_d = nc.dram_tensor("woutt", [H2 + 1, 1], f32, kind="ExternalInput")
    out = nc.dram_tensor("out", [RPC], f32, kind="ExternalOutput")

    with tile.TileContext(nc) as tc:
        with ExitStack() as ctx:
            const = ctx.enter_context(tc.tile_pool(name="const", bufs=1))
            gp = ctx.enter_context(tc.tile_pool(name="g", bufs=1))
            sp = ctx.enter_context(tc.tile_pool(name="s", bufs=1))
            smp = ctx.enter_context(tc.tile_pool(name="small", bufs=1))
            psa = ctx.enter_context(
                tc.tile_pool(name="psa", bufs=1, space="PSUM")
            )
            ps2 = ctx.enter_context(tc.tile_pool(name="ps2", bufs=1, space="PSUM"))
            ps3 = ctx.enter_context(tc.tile_pool(name="ps3", bufs=1, space="PSUM"))

            idxt = const.tile([128, NCH, NPC // 16], i16)
            nc.sync.dma_start(idxt[:], idx_d.ap())
            ridt = const.tile([128, NCH, GPC], bf16)
            nc.sync.dma_start(ridt[:], rid_d.ap())
            iota = const.tile([128, 128], bf16)
            nc.sync.dma_start(iota[:], iota_d.ap())
            b1_s = const.tile([128, MH], f32)
            nc.sync.dma_start(b1_s[:], b1_d.ap())
            w2t_s = const.tile([128, MH, H2], f32)
            nc.scalar.dma_start(w2t_s[:], w2t_d.ap())
            b2_s = const.tile([H2, 1], f32)
            nc.scalar.dma_start(b2_s[:], b2_d.ap())
            wout_s = const.tile([H2 + 1, 1], f32)
            nc.scalar.dma_start(wout_s[:], wout_d.ap())

            # 3 gathers per table chunk (1024+1024+512 idxs) into one tile:
            # >1024 idxs in one dma_gather overflows the per-queue SWDGE
            # descriptor-ring carveout and deadlocks the ucode's await_space
            # on hardware; 4 queues overlap the latency-bound transfers.
            # slot i -> gt[i % 128, i // 128, :]
            # chunk 0 leads with a small gather so its DMA sem fires early
            # and the PE (end-critical) starts ~9us sooner
            CSPLITS = {0: (4, 8, 6), 1: (8, 8, 2), 2: (8, 8, 2), 3: (8, 8, 2)}
            COFF = {c: (0, s[0], s[0] + s[1]) for c, s in CSPLITS.items()}
            gts = {}               # (chunk, split) -> tile, for fine deps:
            nq = 0                 # matmuls start when their split lands
            for c in range(NCH):
                g0 = 0
                for v, sg in enumerate(CSPLITS[c]):
                    gt = gp.tile([128, sg, H1], bf16, name=f"g{c}_{v}",
                                 tag=f"g{c}_{v}")
                    nc.gpsimd.dma_gather(
                        gt[:],
                        table.ap()[c],
                        idxt[:, c, g0 * 8:(g0 + sg) * 8],
                        sg * 128,
                        sg * 128,
                        H1,
                        queue_num=nq % 4,
                    )
                    nq += 1
                    g0 += sg
                    gts[c, v] = gt

            # selection matrices S[c,b][slot, g, row] = (rowid == row)
            sts = {}
            for c in range(NCH):
                for b in range(BANDS):
                    st = sp.tile([128, GPB, 128], bf16, name=f"s{c}_{b}", tag=f"s{c}_{b}")
                    nc.vector.scalar_tensor_tensor(
                        st[:],
                        ridt[:, c, b * GPB:(b + 1) * GPB]
                        .unsqueeze(2)
                        .broadcast_to([128, GPB, 128]),
                        0.0,
                        iota[:].unsqueeze(1).broadcast_to([128, GPB, 128]),
                        mybir.AluOpType.add,
                        mybir.AluOpType.is_equal,
                    )
                    sts[c, b] = st

            # fc1: psum[b][h][emb, row] += G[slot, emb].T @ S[slot, row]
            psum = [
                [psa.tile([128, 128], f32, name=f"ps{b}_{h}") for h in range(MH)]
                for b in range(BANDS)
            ]
            for c in range(NCH):
                for b in range(BANDS):
                    for g in range(GPB):
                        gg = b * GPB + g          # slot group within chunk
                        off = COFF[c]
                        v = 0 if gg < off[1] else (1 if gg < off[2] else 2)
                        for h in range(MH):
                            nc.tensor.matmul(
                                psum[b][h][:],
                                gts[c, v][:, gg - off[v],
                                          h * 128:(h + 1) * 128],
                                sts[c, b][:, g, :],
                                start=(c == 0 and g == 0),
                                stop=(c == NCH - 1 and g == GPB - 1),
                            )

            # per-band tail: relu+bias (h1 already [emb-part, row]), fc2,
            # relu+b2 — band 0's tail overlaps band 1's last fc1 matmuls
            h1t = smp.tile([128, MH, RPC], f32, name="h1t")
            p2 = ps2.tile([H2, RPC], f32, name="p2")
            h2t = smp.tile([H2 + 1, RPC], f32, name="h2t")
            for b in range(BANDS):
                for h in range(MH):
                    nc.scalar.activation(
                        h1t[:, h, b * 128:(b + 1) * 128],
                        psum[b][h][:],
                        mybir.ActivationFunctionType.Relu,
                        bias=b1_s[:, h:h + 1],
                    )
                for h in range(MH):
                    nc.tensor.matmul(
                        p2[:, b * 128:(b + 1) * 128],
                        w2t_s[:, h, :],
                        h1t[:, h, b * 128:(b + 1) * 128],
                        start=(h == 0), stop=(h == MH - 1),
                    )
                nc.scalar.activation(
                    h2t[0:H2, b * 128:(b + 1) * 128],
                    p2[:, b * 128:(b + 1) * 128],
                    mybir.ActivationFunctionType.Relu,
                    bias=b2_s[:],
                )
            nc.vector.memset(h2t[H2:H2 + 1, :], 1.0)

            # fc3 (bout folded in via the ones row)
            p3 = ps3.tile([1, RPC], f32, name="p3")
            nc.tensor.matmul(p3[:], wout_s[:], h2t[:], start=True, stop=True)
            ot = smp.tile([1, RPC], f32, name="ot")
            nc.vector.tensor_copy(ot[:], p3[:])
            nc.sync.dma_start(out.ap(), ot[:])

    nc.compile()
    return nc


def get_program():
    if "nc" not in _CACHED:
        _CACHED["nc"] = _build_program()
    return _CACHED["nc"]


def _prep_inputs(x, W1, b1, W2, b2, Wout, bout):
    """Convert the dense few-hot x into per-core gather index lists and build
    the shared bf16 embedding table + small fc weights."""
    bf = ml_dtypes.bfloat16

    w1T = np.ascontiguousarray(W1.T).astype(bf)             # [IN_DIM, H1]
    table = np.zeros((NCH, CHR + 1, H1), dtype=bf)
    table[:, :CHR, :] = w1T.reshape(NCH, CHR, H1)

    b1_h = np.ascontiguousarray(b1.reshape(MH, 128).T)      # [128, MH]
    w2t_h = np.ascontiguousarray(
        W2.T.reshape(MH, 128, H2).transpose(1, 0, 2)        # [128, MH, H2]
    )
    b2_h = np.ascontiguousarray(b2.reshape(H2, 1)).astype(np.float32)
    wout_h = np.concatenate(
        [Wout.T, bout.reshape(1, 1)], axis=0
    ).astype(np.float32)                                    # [H2+1, 1]
    iota_h = np.ascontiguousarray(
        np.broadcast_to(np.arange(128, dtype=np.float32), (128, 128))
    ).astype(bf)

    rows, cols = np.nonzero(x)                              # row-major sorted
    in_maps = []
    for cidx in range(N_CORES):
        m = (rows >= cidx * RPC) & (rows < (cidx + 1) * RPC)
        r = rows[m] - cidx * RPC
        f = cols[m]
        ch = f // CHR
        band = r // 128

        idx_arr = np.full((NCH, NPC), ZROW, dtype=np.int16)
        rid_arr = np.full((NCH, GPC, 128), -1.0, dtype=np.float32)
        for c in range(NCH):
            for b in range(BANDS):
                sel = (ch == c) & (band == b)
                n = int(sel.sum())
                assert n <= NPB, f"slot padding overflow: {n} > {NPB}"
                # ascending feature order -> DMA descriptors walk increasing
                # HBM addresses (S reassigns slots to rows, any order works)
                order = np.argsort(f[sel], kind="stable")
                pos = b * NPB + np.arange(n)
                idx_arr[c, pos] = (f[sel][order] - c * CHR).astype(np.int16)
                rid_arr[c, pos // 128, pos % 128] = r[sel][order] - b * 128

        # dma_gather reads slot i's index at idxs[i % 16, i // 16], replicated
        # across the eight 16-partition gpsimd cores
        w = idx_arr.reshape(NCH, NPC // 16, 16)             # [c, s, j]
        idx_t = np.ascontiguousarray(
            np.tile(w.transpose(2, 0, 1), (8, 1, 1))        # [128, c, s]
        )
        rid_t = np.ascontiguousarray(
            rid_arr.transpose(2, 0, 1).astype(bf)           # [128, NCH, GPC]
        )
        in_maps.append({
            "table": table,
            "idx": idx_t,
            "rid": rid_t,
            "iota": iota_h,
            "b1": b1_h,
            "w2t": w2t_h,
            "b2": b2_h,
            "woutt": wout_h,
        })
    return in_maps


def kernel(x, W1, b1, W2, b2, Wout, bout, _trace=False, _trace_kwargs=None):
    x = np.asarray(x, dtype=np.float32)
    W1 = np.asarray(W1, dtype=np.float32)
    b1 = np.asarray(b1, dtype=np.float32)
    W2 = np.asarray(W2, dtype=np.float32)
    b2 = np.asarray(b2, dtype=np.float32)
    Wout = np.asarray(Wout, dtype=np.float32)
    bout = np.asarray(bout, dtype=np.float32)

    nc = get_program()
    in_maps = _prep_inputs(x, W1, b1, W2, b2, Wout, bout)
    res = run_bass_kernel_spmd(
        nc,
        in_maps,
        core_ids=list(range(N_CORES)),
        trace=_trace,
        **(_trace_kwargs or {}),
    )
    out = np.concatenate(
        [res.results[c]["out"] for c in range(N_CORES)]
    ).reshape(B, 1).astype(np.float32)
    if _trace:
        kernel.last_results = res
    return out


if __name__ == "__main__":
    # quick self-run with random data (not the reference distribution)
    rng = np.random.default_rng(0)
    x = (rng.random((B, IN_DIM)) < 32.0 / IN_DIM).astype(np.float32)
    W1 = rng.standard_normal((H1, IN_DIM), dtype=np.float32) / np.sqrt(IN_DIM)
    b1 = rng.standard_normal(H1, dtype=np.float32) / np.sqrt(IN_DIM)
    W2 = rng.standard_normal((H2, H1), dtype=np.float32) / np.sqrt(H1)
    b2 = rng.standard_normal(H2, dtype=np.float32) / np.sqrt(H1)
    Wout = rng.standard_normal((1, H2), dtype=np.float32) / np.sqrt(H2)
    bout = rng.standard_normal(1, dtype=np.float32) / np.sqrt(H2)
    got = kernel(x, W1, b1, W2, b2, Wout, bout)
    h1 = np.maximum(x @ W1.T + b1, 0)
    h2 = np.maximum(h1 @ W2.T + b2, 0)
    exp = h2 @ Wout.T + bout
    print("rel err:", np.abs(got - exp).max() / np.abs(exp).max())


# revision 26
# speedup vs baseline: 7.1274x; 1.0181x over previous
"""HalfKA NNUE forward pass on 8 Trainium2 NeuronCores — sparse gather version.

Network (fp32 reference):
    h1  = relu(x @ W1.T + b1)     x:[2048, 98304] sparse 0/1, W1:[256, 98304]
    h2  = relu(h1 @ W2.T + b2)    W2:[32, 256]
    out = h2 @ Wout.T + bout      Wout:[1, 32]  -> [2048, 1]

x is a few-hot mask (~32 active features per row), so fc1 is an embedding
lookup: h1[b] = sum_{i in active(b)} W1[:, i] + b1. Instead of streaming the
805 MB dense x, the host converts each row to its active-index list and the
device gathers the corresponding 256-dim bf16 embedding columns straight from
HBM with dma_gather (~0.5 MB/core of random 512 B reads).

Sharding: data-parallel over batch — core c owns rows [256c, 256(c+1)), no
collectives. The bf16 embedding table (W1.T) is replicated in every core's
DRAM, split into 4 chunks of 24576 rows (+1 zero pad row each) because
dma_gather indices are int16.

Per core, slots are grouped per (chunk, band-of-128-rows) and padded to a
fixed 1280 (actual max 1116) with zero-row pads. The gathered block
G[slot, emb] for each 128-slot group is reduced into per-row h1 on the PE:
      h1T[emb, row] += G[:, emb].T @ S[:, row]
where S[slot, row] = (rowid[slot] == row) is a one-hot selection matrix built
on the DVE from host-shipped row ids (pads get rowid -1 => zero column). The
result lands directly in the [emb-partition, batch-free] layout that fc2
wants, so bias+relu is a single activation per psum tile and fc2/fc3 are the
same tiny matmuls as the dense kernel.
"""

import sys

sys.path.insert(0, "/opt/trn_rl_repo")

from contextlib import ExitStack

import numpy as np
import ml_dtypes

import concourse.bass as bass  # noqa: F401  (registers engine libraries)
import concourse.tile as tile
from concourse import bacc, mybir
from concourse.bass_utils import run_bass_kernel_spmd

f32 = mybir.dt.float32
bf16 = mybir.dt.bfloat16
i16 = mybir.dt.int16

N_CORES = 8
B = 2048
IN_DIM = 98304
H1 = 256
H2 = 32

RPC = B // N_CORES      # 256 rows per core
BANDS = 2               # 128-row PE bands per core
NCH = 4                 # embedding-table chunks (int16 index range)
CHR = IN_DIM // NCH     # 24576 feature rows per chunk
ZROW = CHR              # zero row appended at the end of each chunk
NPB = 1152              # padded slots per (chunk, band); actual max 1116
GPB = NPB // 128        # 10 groups of 128 slots per band
NPC = NPB * BANDS       # 2560 slots per chunk-gather
GPC = GPB * BANDS       # 20 groups per chunk
MH = H1 // 128          # 2 psum halves of the 256-dim h1

_CACHED = {}


def _build_program():
    nc = bacc.Bacc(
        "TRN2",
        target_bir_lowering=False,
        debug=False,
        num_devices=N_CORES,
        num_swdge_queues=4,
    )

    table = nc.dram_tensor("table", [NCH, CHR + 1, H1], bf16, kind="ExternalInput")
    idx_d = nc.dram_tensor("idx", [128, NCH, NPC // 16], i16, kind="ExternalInput")
    rid_d = nc.dram_tensor("rid", [128, NCH, GPC], bf16, kind="ExternalInput")
    iota_d = nc.dram_tensor("iota", [128, 128], bf16, kind="ExternalInput")
    b1_d = nc.dram_tensor("b1", [128, MH], f32, kind="ExternalInput")
    w2t_d = nc.dram_tensor("w2t", [128, MH, H2], f32, kind="ExternalInput")
    b2_d = nc.dram_tensor("b2", [H2, 1], f32, kind="ExternalInput")
    wout_d = nc.dram_tensor("woutt", [H2 + 1, 1], f32, kind="ExternalInput")
    out = nc.dram_tensor("out", [RPC], f32, kind="ExternalOutput")

    with tile.TileContext(nc) as tc:
        with ExitStack() as ctx:
            const = ctx.enter_context(tc.tile_pool(name="const", bufs=1))
            gp = ctx.enter_context(tc.tile_pool(name="g", bufs=1))
            sp = ctx.enter_context(tc.tile_pool(name="s", bufs=1))
            smp = ctx.enter_context(tc.tile_pool(name="small", bufs=1))
            psa = ctx.enter_context(
                tc.tile_pool(name="psa", bufs=1, space="PSUM")
            )
            ps2 = ctx.enter_context(tc.tile_pool(name="ps2", bufs=1, space="PSUM"))
            ps3 = ctx.enter_context(tc.tile_pool(name="ps3", bufs=1, space="PSUM"))

            idxt = const.tile([128, NCH, NPC // 16], i16)
            nc.sync.dma_start(idxt[:], idx_d.ap())
            ridt = const.tile([128, NCH, GPC], bf16)
            nc.sync.dma_start(ridt[:], rid_d.ap())
            iota = const.tile([128, 128], bf16)
            nc.sync.dma_start(iota[:], iota_d.ap())
            b1_s = const.tile([128, MH], f32)
            nc.sync.dma_start(b1_s[:], b1_d.ap())
            w2t_s = const.tile([128, MH, H2], f32)
            nc.scalar.dma_start(w2t_s[:], w2t_d.ap())
            b2_s = const.tile([H2, 1], f32)
            nc.scalar.dma_start(b2_s[:], b2_d.ap())
            wout_s = const.tile([H2 + 1, 1], f32)
            nc.scalar.dma_start(wout_s[:], wout_d.ap())

            # 3 gathers per table chunk (1024+1024+512 idxs) into one tile:
            # >1024 idxs in one dma_gather overflows the per-queue SWDGE
            # descriptor-ring carveout and deadlocks the ucode's await_space
            # on hardware; 4 queues overlap the latency-bound transfers.
            # slot i -> gt[i % 128, i // 128, :]
            # chunk 0 leads with a small gather so its DMA sem fires early
            # and the PE (end-critical) starts ~9us sooner
            CSPLITS = {0: (4, 8, 6), 1: (8, 8, 2), 2: (8, 8, 2), 3: (8, 8, 2)}
            COFF = {c: (0, s[0], s[0] + s[1]) for c, s in CSPLITS.items()}
            # queue totals balanced to 2304 slots each (round-robin loads one
            # queue with 2816 and the slowest queue sets the gather-phase end)
            QASSIGN = (0, 1, 0, 2, 3, 1, 0, 2, 3, 1, 3, 2)
            gts = {}               # (chunk, split) -> tile, for fine deps:
            nq = 0                 # matmuls start when their split lands
            for c in range(NCH):
                g0 = 0
                for v, sg in enumerate(CSPLITS[c]):
                    gt = gp.tile([128, sg, H1], bf16, name=f"g{c}_{v}",
                                 tag=f"g{c}_{v}")
                    nc.gpsimd.dma_gather(
                        gt[:],
                        table.ap()[c],
                        idxt[:, c, g0 * 8:(g0 + sg) * 8],
                        sg * 128,
                        sg * 128,
                        H1,
                        queue_num=QASSIGN[nq],
                    )
                    nq += 1
                    g0 += sg
                    gts[c, v] = gt

            # selection matrices S[c,b][slot, g, row] = (rowid == row)
            sts = {}
            for c in range(NCH):
                for b in range(BANDS):
                    st = sp.tile([128, GPB, 128], bf16, name=f"s{c}_{b}", tag=f"s{c}_{b}")
                    nc.vector.scalar_tensor_tensor(
                        st[:],
                        ridt[:, c, b * GPB:(b + 1) * GPB]
                        .unsqueeze(2)
                        .broadcast_to([128, GPB, 128]),
                        0.0,
                        iota[:].unsqueeze(1).broadcast_to([128, GPB, 128]),
                        mybir.AluOpType.add,
                        mybir.AluOpType.is_equal,
                    )
                    sts[c, b] = st

            # fc1: psum[b][h][emb, row] += G[slot, emb].T @ S[slot, row]
            psum = [
                [psa.tile([128, 128], f32, name=f"ps{b}_{h}") for h in range(MH)]
                for b in range(BANDS)
            ]
            for c in range(NCH):
                for b in range(BANDS):
                    for g in range(GPB):
                        gg = b * GPB + g          # slot group within chunk
                        off = COFF[c]
                        v = 0 if gg < off[1] else (1 if gg < off[2] else 2)
                        for h in range(MH):
                            nc.tensor.matmul(
                                psum[b][h][:],
                                gts[c, v][:, gg - off[v],
                                          h * 128:(h + 1) * 128],
                                sts[c, b][:, g, :],
                                start=(c == 0 and g == 0),
                                stop=(c == NCH - 1 and g == GPB - 1),
                            )

            # per-band tail: relu+bias (h1 already [emb-part, row]), fc2,
            # relu+b2 — band 0's tail overlaps band 1's last fc1 matmuls
            h1t = smp.tile([128, MH, RPC], f32, name="h1t")
            p2 = ps2.tile([H2, RPC], f32, name="p2")
            h2t = smp.tile([H2 + 1, RPC], f32, name="h2t")
            for b in range(BANDS):
                for h in range(MH):
                    nc.scalar.activation(
                        h1t[:, h, b * 128:(b + 1) * 128],
                        psum[b][h][:],
                        mybir.ActivationFunctionType.Relu,
                        bias=b1_s[:, h:h + 1],
                    )
                for h in range(MH):
                    nc.tensor.matmul(
                        p2[:, b * 128:(b + 1) * 128],
                        w2t_s[:, h, :],
                        h1t[:, h, b * 128:(b + 1) * 128],
                        start=(h == 0), stop=(h == MH - 1),
                    )
                nc.scalar.activation(
                    h2t[0:H2, b * 128:(b + 1) * 128],
                    p2[:, b * 128:(b + 1) * 128],
                    mybir.ActivationFunctionType.Relu,
                    bias=b2_s[:],
                )
            nc.vector.memset(h2t[H2:H2 + 1, :], 1.0)

            # fc3 (bout folded in via the ones row)
            p3 = ps3.tile([1, RPC], f32, name="p3")
            nc.tensor.matmul(p3[:], wout_s[:], h2t[:], start=True, stop=True)
            ot = smp.tile([1, RPC], f32, name="ot")
            nc.vector.tensor_copy(ot[:], p3[:])
            nc.sync.dma_start(out.ap(), ot[:])

    nc.compile()
    return nc


def get_program():
    if "nc" not in _CACHED:
        _CACHED["nc"] = _build_program()
    return _CACHED["nc"]


def _prep_inputs(x, W1, b1, W2, b2, Wout, bout):
    """Convert the dense few-hot x into per-core gather index lists and build
    the shared bf16 embedding table + small fc weights."""
    bf = ml_dtypes.bfloat16

    w1T = np.ascontiguousarray(W1.T).astype(bf)             # [IN_DIM, H1]
    table = np.zeros((NCH, CHR + 1, H1), dtype=bf)
    table[:, :CHR, :] = w1T.reshape(NCH, CHR, H1)

    b1_h = np.ascontiguousarray(b1.reshape(MH, 128).T)      # [128, MH]
    w2t_h = np.ascontiguousarray(
        W2.T.reshape(MH, 128, H2).transpose(1, 0, 2)        # [128, MH, H2]
    )
    b2_h = np.ascontiguousarray(b2.reshape(H2, 1)).astype(np.float32)
    wout_h = np.concatenate(
        [Wout.T, bout.reshape(1, 1)], axis=0
    ).astype(np.float32)                                    # [H2+1, 1]
    iota_h = np.ascontiguousarray(
        np.broadcast_to(np.arange(128, dtype=np.float32), (128, 128))
    ).astype(bf)

    rows, cols = np.nonzero(x)                              # row-major sorted
    in_maps = []
    for cidx in range(N_CORES):
        m = (rows >= cidx * RPC) & (rows < (cidx + 1) * RPC)
        r = rows[m] - cidx * RPC
        f = cols[m]
        ch = f // CHR
        band = r // 128

        idx_arr = np.full((NCH, NPC), ZROW, dtype=np.int16)
        rid_arr = np.full((NCH, GPC, 128), -1.0, dtype=np.float32)
        for c in range(NCH):
            for b in range(BANDS):
                sel = (ch == c) & (band == b)
                n = int(sel.sum())
                assert n <= NPB, f"slot padding overflow: {n} > {NPB}"
                # ascending feature order -> DMA descriptors walk increasing
                # HBM addresses (S reassigns slots to rows, any order works)
                order = np.argsort(f[sel], kind="stable")
                pos = b * NPB + np.arange(n)
                idx_arr[c, pos] = (f[sel][order] - c * CHR).astype(np.int16)
                rid_arr[c, pos // 128, pos % 128] = r[sel][order] - b * 128

        # dma_gather reads slot i's index at idxs[i % 16, i // 16], replicated
        # across the eight 16-partition gpsimd cores
        w = idx_arr.reshape(NCH, NPC // 16, 16)             # [c, s, j]
        idx_t = np.ascontiguousarray(
            np.tile(w.transpose(2, 0, 1), (8, 1, 1))        # [128, c, s]
        )
        rid_t = np.ascontiguousarray(
            rid_arr.transpose(2, 0, 1).astype(bf)           # [128, NCH, GPC]
        )
        in_maps.append({
            "table": table,
            "idx": idx_t,
            "rid": rid_t,
            "iota": iota_h,
            "b1": b1_h,
            "w2t": w2t_h,
            "b2": b2_h,
            "woutt": wout_h,
        })
    return in_maps


def kernel(x, W1, b1, W2, b2, Wout, bout, _trace=False, _trace_kwargs=None):
    x = np.asarray(x, dtype=np.float32)
    W1 = np.asarray(W1, dtype=np.float32)
    b1 = np.asarray(b1, dtype=np.float32)
    W2 = np.asarray(W2, dtype=np.float32)
    b2 = np.asarray(b2, dtype=np.float32)
    Wout = np.asarray(Wout, dtype=np.float32)
    bout = np.asarray(bout, dtype=np.float32)

    nc = get_program()
    in_maps = _prep_inputs(x, W1, b1, W2, b2, Wout, bout)
    res = run_bass_kernel_spmd(
        nc,
        in_maps,
        core_ids=list(range(N_CORES)),
        trace=_trace,
        **(_trace_kwargs or {}),
    )
    out = np.concatenate(
        [res.results[c]["out"] for c in range(N_CORES)]
    ).reshape(B, 1).astype(np.float32)
    if _trace:
        kernel.last_results = res
    return out


if __name__ == "__main__":
    # quick self-run with random data (not the reference distribution)
    rng = np.random.default_rng(0)
    x = (rng.random((B, IN_DIM)) < 32.0 / IN_DIM).astype(np.float32)
    W1 = rng.standard_normal((H1, IN_DIM), dtype=np.float32) / np.sqrt(IN_DIM)
    b1 = rng.standard_normal(H1, dtype=np.float32) / np.sqrt(IN_DIM)
    W2 = rng.standard_normal((H2, H1), dtype=np.float32) / np.sqrt(H1)
    b2 = rng.standard_normal(H2, dtype=np.float32) / np.sqrt(H1)
    Wout = rng.standard_normal((1, H2), dtype=np.float32) / np.sqrt(H2)
    bout = rng.standard_normal(1, dtype=np.float32) / np.sqrt(H2)
    got = kernel(x, W1, b1, W2, b2, Wout, bout)
    h1 = np.maximum(x @ W1.T + b1, 0)
    h2 = np.maximum(h1 @ W2.T + b2, 0)
    exp = h2 @ Wout.T + bout
    print("rel err:", np.abs(got - exp).max() / np.abs(exp).max())
